# revision 24
# baseline (speedup 1.0000x reference)
"""Causal self-attention on 8 Trainium2 NeuronCores.

Sharding: 2 heads per core (tensor parallel).  The host pre-transposes the
activations/weights into the layouts the PE array wants, each core computes
QKV -> causal attention -> its partial of the output projection for its two
heads, and the host sums the 8 partial projections (row-parallel linear).

Per-core device program (SPMD, different data per core):
  xT    [1024, 4096]  x transposed, rows=embed c, cols=token t (t = b*2048+tt)
  wqkvT [1024, 384]   w_attn rows for this core's heads, transposed.
                      f = [q_h0 d0..63 | q_h1 | k_h0 | k_h1 | v_h0 | v_h1]
  wpT   [128, 1024]   w_proj columns for this core's channels, transposed
  y     [4096, 1024]  partial output (sum over cores = final)

Dataflow (everything "transposed" so the PE contraction dim is the partition
dim with no on-device transposes of activations):
  qkvT[f, t]   = wqkvT_tile.T @ xT_tile            (accumulate over 8 c-tiles)
  S^T[kt, qt]  = kT_tile.T @ qT_block              (K = head dim 64)
  P^T          = exp(S^T / 32)                     (ACT; no max subtraction --
                                                    scores are O(1), exp safe)
  causal mask  = multiply diagonal 128x128 block by 0/1 lower-tri tile
  outT[65,qt] += [V | ones].T @ P^T                (row 64 = softmax sums)
  attnT        = outT[0:64] * (1 / outT[64])       (broadcast along partitions)
  y[t, f]      = attnT_tile.T @ wpT                (partial; host sums cores)

All matmuls run as float32r (fp32 bitcast): 1 PE cycle/row when the moving
free dim is >= 256 -- full bf16-class speed with ~fp22 mantissa precision.
"""

import numpy as np

B, T, C = 2, 2048, 1024
H, D = 16, 64
NCORES = 8
HPC = H // NCORES          # heads per core = 2
BT = B * T                 # 4096 tokens total
TB = 512                   # token block (matmul moving free dim)
CK = C // 128              # 8 contraction tiles for the projections
NTB = BT // TB             # 8 token blocks
NQB = T // TB              # 4 q blocks per batch
NKT = T // 128             # 16 kt tiles per batch
SCALE = 1.0 / 32.0         # 1 / sqrt(C)


def build_program():
    """Build the single-core Bass program (same program runs on all 8 cores)."""
    from contextlib import ExitStack

    import concourse.mybir as mybir
    import concourse.tile as tile
    from concourse import bacc, library_config

    dt = mybir.dt
    F32 = dt.float32
    F32R = dt.float32r

    nc = bacc.Bacc("TRN2")
    xT = nc.dram_tensor("xT", [C, BT], F32, kind="ExternalInput").ap()
    wqkvT = nc.dram_tensor("wqkvT", [C, 3 * HPC * D], F32, kind="ExternalInput").ap()
    wpT = nc.dram_tensor("wpT", [HPC * D, C], F32, kind="ExternalInput").ap()
    # consts[0] = 128x128 identity, consts[1] = causal keep-mask
    # (mask[kt, qt] = 1.0 where kt <= qt)
    consts = nc.dram_tensor("consts", [2, 128, 128], F32, kind="ExternalInput").ap()
    y = nc.dram_tensor("y", [BT, C], F32, kind="ExternalOutput").ap()

    with ExitStack() as ctx:
        tc = ctx.enter_context(tile.TileContext(nc))
        const = ctx.enter_context(tc.tile_pool(name="const", bufs=1))
        xpool = ctx.enter_context(tc.tile_pool(name="xload", bufs=12))
        ppool = ctx.enter_context(tc.tile_pool(name="pexp", bufs=4))
        npool = ctx.enter_context(tc.tile_pool(name="norm", bufs=4))
        pvpool = ctx.enter_context(tc.tile_pool(name="pvs", bufs=3))
        ypool = ctx.enter_context(tc.tile_pool(name="yout", bufs=3))
        psA = ctx.enter_context(tc.tile_pool(name="psA", bufs=2, space="PSUM"))
        psPV = ctx.enter_context(tc.tile_pool(name="psPV", bufs=2, space="PSUM"))

        # ---------- constants / persistent SBUF ----------
        w_sb = const.tile([128, CK, 3 * HPC * D], F32R, name="w_sb")
        nc.sync.dma_start(w_sb[:], wqkvT.rearrange("(a p) f -> p a f", p=128).bitcast(F32R))
        wp_sb = const.tile([128, C], F32R, name="wp_sb")
        nc.sync.dma_start(wp_sb[:], wpT.bitcast(F32R))

        ident = const.tile([128, 128], F32R, name="ident")
        nc.sync.dma_start(ident[:], consts[0].bitcast(F32R))
        trimask2 = const.tile([128, HPC, 128], F32, name="trimask2")
        for _h in range(HPC):
            nc.sync.dma_start(trimask2[:, _h, :], consts[1])
        # partition_broadcast lives in the "attn" GPSIMD library; same-engine
        # FIFO order guarantees this lands before the broadcasts.
        nc.gpsimd.load_library(library_config.attn)

        # Per-batch transposed activations, heads packed on partitions
        # (h0 -> partitions 0:64, h1 -> 64:128).
        qT = [const.tile([128, T], F32R, name=f"qT{b}") for b in range(B)]
        kT = [const.tile([128, T], F32R, name=f"kT{b}") for b in range(B)]
        vT = [const.tile([128, T], F32R, name=f"vT{b}") for b in range(B)]
        attnT = [const.tile([128, T], F32R, name=f"attnT{b}") for b in range(B)]

        # [V | ones] stationary tiles for PV: V1[:, b, h, kti, 0:64] = V natural
        # [kt, d]; column 64 = 1.0 so PV row 64 accumulates the softmax sums.
        V1 = const.tile([128, B, HPC, NKT, 65], F32R, name="V1")
        nc.vector.memset(V1[:, :, :, :, 64:65].bitcast(F32), 1.0)

        # ---------- phase 1: QKV projection ----------
        dest = {0: qT, 1: kT, 2: vT}
        for tb in range(NTB):
            b, tcol = divmod(tb, NTB // B)
            xts = []
            for ci in range(CK):
                xt = xpool.tile([128, TB], F32R, name="xt", tag="xt")
                nc.sync.dma_start(
                    xt[:],
                    xT[ci * 128 : (ci + 1) * 128, tb * TB : (tb + 1) * TB].bitcast(F32R),
                )
                xts.append(xt)
            qtags = ["psA", "psPV0", "psPV1"]
            pss = [
                psA.tile([128, TB], F32, name="qkv_ps", tag="psA")
                if fi == 0
                else psPV.tile([128, TB], F32, name=f"qkv_ps{fi}", tag=qtags[fi])
                for fi in range(3)
            ]
            for ci in range(CK):
                for fi in range(3):
                    nc.tensor.matmul(
                        pss[fi][:],
                        w_sb[:, ci, fi * 128 : (fi + 1) * 128],
                        xts[ci][:],
                        start=(ci == 0),
                        stop=(ci == CK - 1),
                    )
            for fi in range(3):
                nc.scalar.copy(
                    out=dest[fi][b][:, tcol * TB : (tcol + 1) * TB], in_=pss[fi][:]
                )

            # As soon as a batch's vT is complete, build its V-natural tiles
            # (PE transpose of 64-row slices through the identity).
            if tcol == NTB // B - 1:
                for h in range(HPC):
                    hp = slice(h * 64, (h + 1) * 64)
                    for kti in range(NKT):
                        tr = psA.tile([128, 64], F32R, name="vtr", tag="psA")
                        nc.tensor.transpose(
                            tr[:], vT[b][hp, kti * 128 : (kti + 1) * 128], ident[hp, hp]
                        )
                        nc.vector.tensor_copy(out=V1[:, b, h, kti, 0:64], in_=tr[:])

        # ---------- phase 2: causal attention ----------
        # Both heads interleaved per (b, qb) and PV software-pipelined one kt
        # tile behind the scores so the PE never stalls on the ACT exp.
        # Unnormalized [PV | sums] results are copied to SBUF (freeing PSUM)
        # and all 16 sum-rows are collected so one batched reciprocal covers
        # the whole kernel (a [1, N] DVE reciprocal is ~3.4 us — single lane).
        for b in range(B):
            for qb in range(NQB):
                nkt = 4 * qb + 4
                pv = [
                    psPV.tile([65, TB], F32, name=f"pv_ps{h}", tag=f"psPV{h}")
                    for h in range(HPC)
                ]
                stages = []  # deferred PV matmuls, one kti behind the scores

                def flush(n=None):
                    while stages and (n is None or len(stages) > n):
                        stages.pop(0)()

                for kti in range(nkt):
                    qs = max(0, kti * 128 - qb * TB)  # local col start
                    N = TB - qs
                    # both heads' scores in one 2-bank PSUM tile -> one exp
                    sps = psA.tile([128, HPC, TB], F32, name="s_ps", tag="psA")
                    for h in range(HPC):
                        hp = slice(h * 64, (h + 1) * 64)
                        nc.tensor.matmul(
                            sps[:, h, 0:N],
                            kT[b][hp, kti * 128 : (kti + 1) * 128],
                            qT[b][hp, qb * TB + qs : (qb + 1) * TB],
                            start=True,
                            stop=True,
                        )
                    P = ppool.tile([128, HPC, TB], F32R, name="Pt", tag="P")
                    nc.scalar.activation(
                        P[:, :, 0:N],
                        sps[:, :, 0:N],
                        mybir.ActivationFunctionType.Exp,
                        scale=SCALE,
                    )
                    if kti * 128 >= qb * TB:
                        # diagonal tile: first 128 cols of each head hold the
                        # triangle; one DVE mult covers both heads
                        nc.vector.tensor_mul(
                            P[:, :, 0:128], P[:, :, 0:128], trimask2[:]
                        )

                    def pv_step(kti=kti, qs=qs, N=N, P=P):
                        for h in range(HPC):
                            nc.tensor.matmul(
                                pv[h][:, qs:TB],
                                V1[:, b, h, kti, :],
                                P[:, h, 0:N],
                                start=(kti == 0),
                                stop=(kti == nkt - 1),
                            )

                    stages.append(pv_step)
                    flush(1)
                flush()

                # normalize this q-block inline (reciprocal_approx_fast is
                # ~18-bit accurate, plenty above the fp32r noise floor), then
                # emit its projection: the proj matmuls are exp-independent
                # PE work that fills the next q-block's ACT stalls.
                for h in range(HPC):
                    hp = slice(h * 64, (h + 1) * 64)
                    pvt = pvpool.tile([65, TB], F32, name="pvt", tag="pvt")
                    nc.vector.tensor_copy(out=pvt[:], in_=pv[h][:])
                    # custom-DVE ops require partition-0 sources on HW; plain
                    # copies handle the 64->0 partition shift fine.
                    s0 = npool.tile([1, TB], F32, name="s0", tag="s0")
                    nc.vector.tensor_copy(out=s0[:], in_=pvt[64:65, :])
                    rt = npool.tile([1, TB], F32, name="rt", tag="rt")
                    nc.vector.reciprocal_approx_fast(rt[:], s0[:])
                    bc = npool.tile([64, TB], F32, name="bc", tag="bc")
                    nc.gpsimd.partition_broadcast(bc[:], rt[:])
                    nc.vector.tensor_mul(
                        attnT[b][hp, qb * TB : (qb + 1) * TB], pvt[0:64, :], bc[:]
                    )
                for ti in range(4 * qb, 4 * qb + 4):
                    for fb in range(C // TB):
                        ps = psA.tile([128, TB], F32, name="y_ps", tag="psA")
                        nc.tensor.matmul(
                            ps[:],
                            attnT[b][:, ti * 128 : (ti + 1) * 128],
                            wp_sb[:, fb * TB : (fb + 1) * TB],
                            start=True,
                            stop=True,
                        )
                        ysb = ypool.tile([128, TB], F32, name="ysb", tag="ysb")
                        nc.vector.tensor_copy(out=ysb[:], in_=ps[:])
                        nc.sync.dma_start(
                            y[b * T + ti * 128 : b * T + (ti + 1) * 128,
                              fb * TB : (fb + 1) * TB],
                            ysb[:],
                        )
    nc.compile()
    return nc


def make_in_maps(x, w_attn, w_proj):
    """Host-side sharding into the per-core layouts."""
    x = np.asarray(x, dtype=np.float32)
    w_attn = np.asarray(w_attn, dtype=np.float32)
    w_proj = np.asarray(w_proj, dtype=np.float32)

    xT = np.ascontiguousarray(x.reshape(BT, C).T)           # [1024, 4096]
    wpT_full = np.ascontiguousarray(w_proj.T)               # [c_in, f_out]

    in_maps = []
    for c in range(NCORES):
        rows = []
        for sec in range(3):                                # q, k, v
            for h in (HPC * c, HPC * c + 1):
                rows.extend(range(sec * C + h * D, sec * C + (h + 1) * D))
        wqkvT = np.ascontiguousarray(w_attn[rows, :].T)     # [1024, 384]
        wpT = np.ascontiguousarray(
            wpT_full[c * HPC * D : (c + 1) * HPC * D, :]    # [128, 1024]
        )
        consts = np.stack(
            [
                np.eye(128, dtype=np.float32),
                np.tril(np.ones((128, 128), np.float32)).T,  # keep kt <= qt
            ]
        )
        in_maps.append({"xT": xT, "wqkvT": wqkvT, "wpT": wpT, "consts": consts})
    return in_maps


_PROGRAM = None


def _program():
    global _PROGRAM
    if _PROGRAM is None:
        _PROGRAM = build_program()
    return _PROGRAM


def kernel(x, w_attn, w_proj):
    from concourse.bass_utils import run_bass_kernel_spmd

    res = run_bass_kernel_spmd(
        _program(), make_in_maps(x, w_attn, w_proj), list(range(NCORES))
    )
    out = res.results[0]["y"].astype(np.float32, copy=True)
    for i in range(1, NCORES):
        out += res.results[i]["y"]
    return out.reshape(B, T, C)


# revision 25
# speedup vs baseline: 1.0644x; 1.0644x over previous
"""Causal self-attention on 8 Trainium2 NeuronCores.

Sharding: 2 heads per core (tensor parallel).  The host pre-transposes the
activations/weights into the layouts the PE array wants, each core computes
QKV -> causal attention -> its partial of the output projection for its two
heads, and the host sums the 8 partial projections (row-parallel linear).

Per-core device program (SPMD, different data per core):
  xT    [1024, 4096]  x transposed, rows=embed c, cols=token t (t = b*2048+tt)
  wqkvT [1024, 384]   w_attn rows for this core's heads, transposed.
                      f = [q_h0 d0..63 | q_h1 | k_h0 | k_h1 | v_h0 | v_h1]
  wpT   [128, 1024]   w_proj columns for this core's channels, transposed
  y     [4096, 1024]  partial output (sum over cores = final)

Dataflow (everything "transposed" so the PE contraction dim is the partition
dim with no on-device transposes of activations):
  qkvT[f, t]   = wqkvT_tile.T @ xT_tile            (accumulate over 8 c-tiles)
  S^T[kt, qt]  = kT_tile.T @ qT_block              (K = head dim 64)
  P^T          = exp(S^T / 32)                     (ACT; no max subtraction --
                                                    scores are O(1), exp safe)
  causal mask  = multiply diagonal 128x128 block by 0/1 lower-tri tile
  outT[65,qt] += [V | ones].T @ P^T                (row 64 = softmax sums)
  attnT        = outT[0:64] * (1 / outT[64])       (broadcast along partitions)
  y[t, f]      = attnT_tile.T @ wpT                (partial; host sums cores)

All matmuls run as float32r (fp32 bitcast): 1 PE cycle/row when the moving
free dim is >= 256 -- full bf16-class speed with ~fp22 mantissa precision.
"""

import numpy as np

B, T, C = 2, 2048, 1024
H, D = 16, 64
NCORES = 8
HPC = H // NCORES          # heads per core = 2
BT = B * T                 # 4096 tokens total
TB = 512                   # token block (matmul moving free dim)
CK = C // 128              # 8 contraction tiles for the projections
NTB = BT // TB             # 8 token blocks
NQB = T // TB              # 4 q blocks per batch
NKT = T // 128             # 16 kt tiles per batch
SCALE = 1.0 / 32.0         # 1 / sqrt(C)


def build_program():
    """Build the single-core Bass program (same program runs on all 8 cores)."""
    from contextlib import ExitStack

    import concourse.mybir as mybir
    import concourse.tile as tile
    from concourse import bacc, library_config

    dt = mybir.dt
    F32 = dt.float32
    F32R = dt.float32r

    nc = bacc.Bacc("TRN2")
    xT = nc.dram_tensor("xT", [C, BT], F32, kind="ExternalInput").ap()
    wqkvT = nc.dram_tensor("wqkvT", [C, 3 * HPC * D], F32, kind="ExternalInput").ap()
    wpT = nc.dram_tensor("wpT", [HPC * D, C], F32, kind="ExternalInput").ap()
    # consts[0] = 128x128 identity, consts[1] = causal keep-mask
    # (mask[kt, qt] = 1.0 where kt <= qt)
    consts = nc.dram_tensor("consts", [2, 128, 128], F32, kind="ExternalInput").ap()
    y = nc.dram_tensor("y", [BT, C], F32, kind="ExternalOutput").ap()

    with ExitStack() as ctx:
        tc = ctx.enter_context(tile.TileContext(nc))
        const = ctx.enter_context(tc.tile_pool(name="const", bufs=1))
        xpool = ctx.enter_context(tc.tile_pool(name="xload", bufs=12))
        ppool = ctx.enter_context(tc.tile_pool(name="pexp", bufs=4))
        npool = ctx.enter_context(tc.tile_pool(name="norm", bufs=4))
        pvpool = ctx.enter_context(tc.tile_pool(name="pvs", bufs=3))
        ypool = ctx.enter_context(tc.tile_pool(name="yout", bufs=3))
        psA = ctx.enter_context(tc.tile_pool(name="psA", bufs=2, space="PSUM"))
        psPV = ctx.enter_context(tc.tile_pool(name="psPV", bufs=2, space="PSUM"))

        # ---------- constants / persistent SBUF ----------
        w_sb = const.tile([128, CK, 3 * HPC * D], F32R, name="w_sb")
        wqkvT_t = wqkvT.rearrange("(a p) f -> p a f", p=128).bitcast(F32R)
        for ci in range(CK):
            nc.sync.dma_start(w_sb[:, ci, :], wqkvT_t[:, ci, :])
        wp_sb = const.tile([128, C], F32R, name="wp_sb")
        nc.sync.dma_start(wp_sb[:], wpT.bitcast(F32R))

        ident = const.tile([128, 128], F32R, name="ident")
        nc.sync.dma_start(ident[:], consts[0].bitcast(F32R))
        trimask2 = const.tile([128, HPC, 128], F32, name="trimask2")
        for _h in range(HPC):
            nc.sync.dma_start(trimask2[:, _h, :], consts[1])
        # partition_broadcast lives in the "attn" GPSIMD library; same-engine
        # FIFO order guarantees this lands before the broadcasts.
        nc.gpsimd.load_library(library_config.attn)

        # Per-batch transposed activations, heads packed on partitions
        # (h0 -> partitions 0:64, h1 -> 64:128).
        qT = [const.tile([128, T], F32R, name=f"qT{b}") for b in range(B)]
        kT = [const.tile([128, T], F32R, name=f"kT{b}") for b in range(B)]
        vT = [const.tile([128, T], F32R, name=f"vT{b}") for b in range(B)]
        attnT = [const.tile([128, T], F32R, name=f"attnT{b}") for b in range(B)]

        # [V | ones] stationary tiles for PV: V1[:, b, h, kti, 0:64] = V natural
        # [kt, d]; column 64 = 1.0 so PV row 64 accumulates the softmax sums.
        V1 = const.tile([128, B, HPC, NKT, 65], F32R, name="V1")
        nc.vector.memset(V1[:, :, :, :, 64:65].bitcast(F32), 1.0)

        # ---------- phase 1: QKV projection ----------
        dest = {0: qT, 1: kT, 2: vT}
        for tb in range(NTB):
            b, tcol = divmod(tb, NTB // B)
            xts = []
            for ci in range(CK):
                xt = xpool.tile([128, TB], F32R, name="xt", tag="xt")
                nc.sync.dma_start(
                    xt[:],
                    xT[ci * 128 : (ci + 1) * 128, tb * TB : (tb + 1) * TB].bitcast(F32R),
                )
                xts.append(xt)
            qtags = ["psA", "psPV0", "psPV1"]
            pss = [
                psA.tile([128, TB], F32, name="qkv_ps", tag="psA")
                if fi == 0
                else psPV.tile([128, TB], F32, name=f"qkv_ps{fi}", tag=qtags[fi])
                for fi in range(3)
            ]
            for ci in range(CK):
                for fi in range(3):
                    nc.tensor.matmul(
                        pss[fi][:],
                        w_sb[:, ci, fi * 128 : (fi + 1) * 128],
                        xts[ci][:],
                        start=(ci == 0),
                        stop=(ci == CK - 1),
                    )
            for fi in range(3):
                nc.scalar.copy(
                    out=dest[fi][b][:, tcol * TB : (tcol + 1) * TB], in_=pss[fi][:]
                )

            # As soon as a batch's vT is complete, build its V-natural tiles
            # (PE transpose of 64-row slices through the identity).
            if tcol == NTB // B - 1:
                for h in range(HPC):
                    hp = slice(h * 64, (h + 1) * 64)
                    for kti in range(NKT):
                        tr = psA.tile([128, 64], F32R, name="vtr", tag="psA")
                        nc.tensor.transpose(
                            tr[:], vT[b][hp, kti * 128 : (kti + 1) * 128], ident[hp, hp]
                        )
                        nc.vector.tensor_copy(out=V1[:, b, h, kti, 0:64], in_=tr[:])

        # ---------- phase 2: causal attention ----------
        # Both heads interleaved per (b, qb) and PV software-pipelined one kt
        # tile behind the scores so the PE never stalls on the ACT exp.
        # Unnormalized [PV | sums] results are copied to SBUF (freeing PSUM)
        # and all 16 sum-rows are collected so one batched reciprocal covers
        # the whole kernel (a [1, N] DVE reciprocal is ~3.4 us — single lane).
        pending_proj = []

        def emit_pending_proj():
            while pending_proj:
                pending_proj.pop(0)()

        for b in range(B):
            for qb in range(NQB):
                nkt = 4 * qb + 4
                pv = [
                    psPV.tile([65, TB], F32, name=f"pv_ps{h}", tag=f"psPV{h}")
                    for h in range(HPC)
                ]
                stages = []  # deferred PV matmuls, one kti behind the scores

                def flush(n=None):
                    while stages and (n is None or len(stages) > n):
                        stages.pop(0)()

                for kti in range(nkt):
                    qs = max(0, kti * 128 - qb * TB)  # local col start
                    N = TB - qs
                    # both heads' scores in one 2-bank PSUM tile -> one exp
                    sps = psA.tile([128, HPC, TB], F32, name="s_ps", tag="psA")
                    for h in range(HPC):
                        hp = slice(h * 64, (h + 1) * 64)
                        nc.tensor.matmul(
                            sps[:, h, 0:N],
                            kT[b][hp, kti * 128 : (kti + 1) * 128],
                            qT[b][hp, qb * TB + qs : (qb + 1) * TB],
                            start=True,
                            stop=True,
                        )
                    P = ppool.tile([128, HPC, TB], F32R, name="Pt", tag="P")
                    nc.scalar.activation(
                        P[:, :, 0:N],
                        sps[:, :, 0:N],
                        mybir.ActivationFunctionType.Exp,
                        scale=SCALE,
                    )
                    if kti * 128 >= qb * TB:
                        # diagonal tile: first 128 cols of each head hold the
                        # triangle; one DVE mult covers both heads
                        nc.vector.tensor_mul(
                            P[:, :, 0:128], P[:, :, 0:128], trimask2[:]
                        )

                    def pv_step(kti=kti, qs=qs, N=N, P=P):
                        for h in range(HPC):
                            nc.tensor.matmul(
                                pv[h][:, qs:TB],
                                V1[:, b, h, kti, :],
                                P[:, h, 0:N],
                                start=(kti == 0),
                                stop=(kti == nkt - 1),
                            )

                    stages.append(pv_step)
                    flush(1)
                    if kti == 2:
                        emit_pending_proj()
                flush()

                # normalize this q-block inline (reciprocal_approx_fast is
                # ~18-bit accurate, plenty above the fp32r noise floor), then
                # emit its projection: the proj matmuls are exp-independent
                # PE work that fills the next q-block's ACT stalls.
                for h in range(HPC):
                    hp = slice(h * 64, (h + 1) * 64)
                    pvt = pvpool.tile([65, TB], F32, name="pvt", tag="pvt")
                    nc.vector.tensor_copy(out=pvt[:], in_=pv[h][:])
                    # custom-DVE ops require partition-0 sources on HW; plain
                    # copies handle the 64->0 partition shift fine.
                    s0 = npool.tile([1, TB], F32, name="s0", tag="s0")
                    nc.vector.tensor_copy(out=s0[:], in_=pvt[64:65, :])
                    rt = npool.tile([1, TB], F32, name="rt", tag="rt")
                    nc.vector.reciprocal_approx_fast(rt[:], s0[:])
                    bc = npool.tile([64, TB], F32, name="bc", tag="bc")
                    nc.gpsimd.partition_broadcast(bc[:], rt[:])
                    nc.vector.tensor_mul(
                        attnT[b][hp, qb * TB : (qb + 1) * TB], pvt[0:64, :], bc[:]
                    )
                def proj_step(b=b, qb=qb):
                    for ti in range(4 * qb, 4 * qb + 4):
                        for fb in range(C // TB):
                            ps = psA.tile([128, TB], F32, name="y_ps", tag="psA")
                            nc.tensor.matmul(
                                ps[:],
                                attnT[b][:, ti * 128 : (ti + 1) * 128],
                                wp_sb[:, fb * TB : (fb + 1) * TB],
                                start=True,
                                stop=True,
                            )
                            ysb = ypool.tile([128, TB], F32, name="ysb", tag="ysb")
                            nc.vector.tensor_copy(out=ysb[:], in_=ps[:])
                            nc.sync.dma_start(
                                y[b * T + ti * 128 : b * T + (ti + 1) * 128,
                                  fb * TB : (fb + 1) * TB],
                                ysb[:],
                            )

                pending_proj.append(proj_step)
        emit_pending_proj()
    nc.compile()
    return nc


def make_in_maps(x, w_attn, w_proj):
    """Host-side sharding into the per-core layouts."""
    x = np.asarray(x, dtype=np.float32)
    w_attn = np.asarray(w_attn, dtype=np.float32)
    w_proj = np.asarray(w_proj, dtype=np.float32)

    xT = np.ascontiguousarray(x.reshape(BT, C).T)           # [1024, 4096]
    wpT_full = np.ascontiguousarray(w_proj.T)               # [c_in, f_out]

    in_maps = []
    for c in range(NCORES):
        rows = []
        for sec in range(3):                                # q, k, v
            for h in (HPC * c, HPC * c + 1):
                rows.extend(range(sec * C + h * D, sec * C + (h + 1) * D))
        wqkvT = np.ascontiguousarray(w_attn[rows, :].T)     # [1024, 384]
        wpT = np.ascontiguousarray(
            wpT_full[c * HPC * D : (c + 1) * HPC * D, :]    # [128, 1024]
        )
        consts = np.stack(
            [
                np.eye(128, dtype=np.float32),
                np.tril(np.ones((128, 128), np.float32)).T,  # keep kt <= qt
            ]
        )
        in_maps.append({"xT": xT, "wqkvT": wqkvT, "wpT": wpT, "consts": consts})
    return in_maps


_PROGRAM = None


def _program():
    global _PROGRAM
    if _PROGRAM is None:
        _PROGRAM = build_program()
    return _PROGRAM


def kernel(x, w_attn, w_proj):
    from concourse.bass_utils import run_bass_kernel_spmd

    res = run_bass_kernel_spmd(
        _program(), make_in_maps(x, w_attn, w_proj), list(range(NCORES))
    )
    out = res.results[0]["y"].astype(np.float32, copy=True)
    for i in range(1, NCORES):
        out += res.results[i]["y"]
    return out.reshape(B, T, C)


# revision 26
# speedup vs baseline: 1.2047x; 1.1317x over previous
"""Causal self-attention on 8 Trainium2 NeuronCores.

Sharding: 2 heads per core (tensor parallel).  The host pre-transposes the
activations/weights into the layouts the PE array wants, each core computes
QKV -> causal attention -> its partial of the output projection for its two
heads, and the host sums the 8 partial projections (row-parallel linear).

Per-core device program (SPMD, different data per core):
  xT    [1024, 4096]  x transposed, rows=embed c, cols=token t (t = b*2048+tt)
  wqkvT [1024, 384]   w_attn rows for this core's heads, transposed.
                      f = [q_h0 d0..63 | q_h1 | k_h0 | k_h1 | v_h0 | v_h1]
  wpT   [128, 1024]   w_proj columns for this core's channels, transposed
  y     [4096, 1024]  partial output (sum over cores = final)

Dataflow (everything "transposed" so the PE contraction dim is the partition
dim with no on-device transposes of activations):
  qkvT[f, t]   = wqkvT_tile.T @ xT_tile            (accumulate over 8 c-tiles)
  S^T[kt, qt]  = kT_tile.T @ qT_block              (K = head dim 64)
  P^T          = exp(S^T / 32)                     (ACT; no max subtraction --
                                                    scores are O(1), exp safe)
  causal mask  = multiply diagonal 128x128 block by 0/1 lower-tri tile
  outT[65,qt] += [V | ones].T @ P^T                (row 64 = softmax sums)
  attnT        = outT[0:64] * (1 / outT[64])       (broadcast along partitions)
  y[t, f]      = attnT_tile.T @ wpT                (partial; host sums cores)

All matmuls run as float32r (fp32 bitcast): 1 PE cycle/row when the moving
free dim is >= 256 -- full bf16-class speed with ~fp22 mantissa precision.
"""

import numpy as np

B, T, C = 2, 2048, 1024
H, D = 16, 64
NCORES = 8
HPC = H // NCORES          # heads per core = 2
BT = B * T                 # 4096 tokens total
TB = 512                   # token block (matmul moving free dim)
CK = C // 128              # 8 contraction tiles for the projections
NTB = BT // TB             # 8 token blocks
NQB = T // TB              # 4 q blocks per batch
NKT = T // 128             # 16 kt tiles per batch
SCALE = 1.0 / 32.0         # 1 / sqrt(C)


def build_program():
    """Build the single-core Bass program (same program runs on all 8 cores)."""
    from contextlib import ExitStack

    import concourse.mybir as mybir
    import concourse.tile as tile
    from concourse import bacc, library_config

    dt = mybir.dt
    F32 = dt.float32
    F32R = dt.float32r
    F16 = dt.float16

    nc = bacc.Bacc("TRN2")
    xT = nc.dram_tensor("xT", [C, BT], F16, kind="ExternalInput").ap()
    wqkvT = nc.dram_tensor("wqkvT", [C, 3 * HPC * D], F16, kind="ExternalInput").ap()
    wpT = nc.dram_tensor("wpT", [HPC * D, C], F16, kind="ExternalInput").ap()
    # consts[0] = 128x128 identity, consts[1] = causal keep-mask
    # (mask[kt, qt] = 1.0 where kt <= qt)
    consts = nc.dram_tensor("consts", [2, 128, 128], F16, kind="ExternalInput").ap()
    y = nc.dram_tensor("y", [BT, C], F32, kind="ExternalOutput").ap()

    with ExitStack() as ctx:
        tc = ctx.enter_context(tile.TileContext(nc))
        const = ctx.enter_context(tc.tile_pool(name="const", bufs=1))
        xpool = ctx.enter_context(tc.tile_pool(name="xload", bufs=12))
        ppool = ctx.enter_context(tc.tile_pool(name="pexp", bufs=4))
        npool = ctx.enter_context(tc.tile_pool(name="norm", bufs=4))
        pvpool = ctx.enter_context(tc.tile_pool(name="pvs", bufs=3))
        ypool = ctx.enter_context(tc.tile_pool(name="yout", bufs=3))
        psA = ctx.enter_context(tc.tile_pool(name="psA", bufs=2, space="PSUM"))
        psPV = ctx.enter_context(tc.tile_pool(name="psPV", bufs=2, space="PSUM"))

        # ---------- constants / persistent SBUF ----------
        w_sb = const.tile([128, CK, 3 * HPC * D], F16, name="w_sb")
        wqkvT_t = wqkvT.rearrange("(a p) f -> p a f", p=128)
        for ci in range(CK):
            nc.sync.dma_start(w_sb[:, ci, :], wqkvT_t[:, ci, :])
        wp_sb = const.tile([128, C], F16, name="wp_sb")
        nc.sync.dma_start(wp_sb[:], wpT)

        ident = const.tile([128, 128], F16, name="ident")
        nc.sync.dma_start(ident[:], consts[0])
        trimask2 = const.tile([128, HPC, 128], F16, name="trimask2")
        for _h in range(HPC):
            nc.sync.dma_start(trimask2[:, _h, :], consts[1])
        # partition_broadcast lives in the "attn" GPSIMD library; same-engine
        # FIFO order guarantees this lands before the broadcasts.
        nc.gpsimd.load_library(library_config.attn)

        # Per-batch transposed activations, heads packed on partitions
        # (h0 -> partitions 0:64, h1 -> 64:128).
        qT = [const.tile([128, T], F16, name=f"qT{b}") for b in range(B)]
        kT = [const.tile([128, T], F16, name=f"kT{b}") for b in range(B)]
        vT = [const.tile([128, T], F16, name=f"vT{b}") for b in range(B)]
        attnT = [const.tile([128, T], F16, name=f"attnT{b}") for b in range(B)]

        # [V | ones] stationary tiles for PV: V1[:, b, h, kti, 0:64] = V natural
        # [kt, d]; column 64 = 1.0 so PV row 64 accumulates the softmax sums.
        V1 = const.tile([128, B, HPC, NKT, 65], F16, name="V1")
        nc.vector.memset(V1[:, :, :, :, 64:65], 1.0)

        # ---------- phase 1: QKV projection ----------
        dest = {0: qT, 1: kT, 2: vT}
        for tb in range(NTB):
            b, tcol = divmod(tb, NTB // B)
            xts = []
            for ci in range(CK):
                xt = xpool.tile([128, TB], F16, name="xt", tag="xt")
                nc.sync.dma_start(
                    xt[:],
                    xT[ci * 128 : (ci + 1) * 128, tb * TB : (tb + 1) * TB],
                )
                xts.append(xt)
            qtags = ["psA", "psPV0", "psPV1"]
            pss = [
                psA.tile([128, TB], F32, name="qkv_ps", tag="psA")
                if fi == 0
                else psPV.tile([128, TB], F32, name=f"qkv_ps{fi}", tag=qtags[fi])
                for fi in range(3)
            ]
            for ci in range(CK):
                for fi in range(3):
                    nc.tensor.matmul(
                        pss[fi][:],
                        w_sb[:, ci, fi * 128 : (fi + 1) * 128],
                        xts[ci][:],
                        start=(ci == 0),
                        stop=(ci == CK - 1),
                    )
            for fi in range(3):
                nc.scalar.copy(
                    out=dest[fi][b][:, tcol * TB : (tcol + 1) * TB], in_=pss[fi][:]
                )

            # As soon as a batch's vT is complete, build its V-natural tiles
            # (PE transpose of 64-row slices through the identity).
            if tcol == NTB // B - 1:
                for h in range(HPC):
                    hp = slice(h * 64, (h + 1) * 64)
                    for kti in range(NKT):
                        tr = psA.tile([128, 64], F16, name="vtr", tag="psA")
                        nc.tensor.transpose(
                            tr[:], vT[b][hp, kti * 128 : (kti + 1) * 128], ident[hp, hp]
                        )
                        nc.vector.tensor_copy(out=V1[:, b, h, kti, 0:64], in_=tr[:])

        # ---------- phase 2: causal attention ----------
        # Both heads interleaved per (b, qb) and PV software-pipelined one kt
        # tile behind the scores so the PE never stalls on the ACT exp.
        # Unnormalized [PV | sums] results are copied to SBUF (freeing PSUM)
        # and all 16 sum-rows are collected so one batched reciprocal covers
        # the whole kernel (a [1, N] DVE reciprocal is ~3.4 us — single lane).
        pending_proj = []

        def emit_pending_proj():
            while pending_proj:
                pending_proj.pop(0)()

        for b in range(B):
            for qb in range(NQB):
                nkt = 4 * qb + 4
                pv = [
                    psPV.tile([65, TB], F32, name=f"pv_ps{h}", tag=f"psPV{h}")
                    for h in range(HPC)
                ]
                stages = []  # deferred PV matmuls, one kti behind the scores

                def flush(n=None):
                    while stages and (n is None or len(stages) > n):
                        stages.pop(0)()

                for kti in range(nkt):
                    qs = max(0, kti * 128 - qb * TB)  # local col start
                    N = TB - qs
                    # both heads' scores in one 2-bank PSUM tile -> one exp
                    sps = psA.tile([128, HPC, TB], F32, name="s_ps", tag="psA")
                    for h in range(HPC):
                        hp = slice(h * 64, (h + 1) * 64)
                        nc.tensor.matmul(
                            sps[:, h, 0:N],
                            kT[b][hp, kti * 128 : (kti + 1) * 128],
                            qT[b][hp, qb * TB + qs : (qb + 1) * TB],
                            start=True,
                            stop=True,
                        )
                    P = ppool.tile([128, HPC, TB], F16, name="Pt", tag="P")
                    nc.scalar.activation(
                        P[:, :, 0:N],
                        sps[:, :, 0:N],
                        mybir.ActivationFunctionType.Exp,
                        scale=SCALE,
                    )
                    if kti * 128 >= qb * TB:
                        # diagonal tile: first 128 cols of each head hold the
                        # triangle; one DVE mult covers both heads
                        nc.vector.tensor_mul(
                            P[:, :, 0:128], P[:, :, 0:128], trimask2[:]
                        )

                    def pv_step(kti=kti, qs=qs, N=N, P=P):
                        for h in range(HPC):
                            nc.tensor.matmul(
                                pv[h][:, qs:TB],
                                V1[:, b, h, kti, :],
                                P[:, h, 0:N],
                                start=(kti == 0),
                                stop=(kti == nkt - 1),
                            )

                    stages.append(pv_step)
                    flush(1)
                    if kti == 2:
                        emit_pending_proj()
                flush()

                # normalize this q-block inline (reciprocal_approx_fast is
                # ~18-bit accurate, plenty above the fp32r noise floor), then
                # emit its projection: the proj matmuls are exp-independent
                # PE work that fills the next q-block's ACT stalls.
                for h in range(HPC):
                    hp = slice(h * 64, (h + 1) * 64)
                    pvt = pvpool.tile([65, TB], F32, name="pvt", tag="pvt")
                    nc.vector.tensor_copy(out=pvt[:], in_=pv[h][:])
                    # custom-DVE ops require partition-0 sources on HW; plain
                    # copies handle the 64->0 partition shift fine.
                    s0 = npool.tile([1, TB], F32, name="s0", tag="s0")
                    nc.vector.tensor_copy(out=s0[:], in_=pvt[64:65, :])
                    rt = npool.tile([1, TB], F32, name="rt", tag="rt")
                    nc.vector.reciprocal_approx_fast(rt[:], s0[:])
                    bc = npool.tile([64, TB], F32, name="bc", tag="bc")
                    nc.gpsimd.partition_broadcast(bc[:], rt[:])
                    nc.vector.tensor_mul(
                        attnT[b][hp, qb * TB : (qb + 1) * TB], pvt[0:64, :], bc[:]
                    )
                def proj_step(b=b, qb=qb):
                    for ti in range(4 * qb, 4 * qb + 4):
                        for fb in range(C // TB):
                            ps = psA.tile([128, TB], F32, name="y_ps", tag="psA")
                            nc.tensor.matmul(
                                ps[:],
                                attnT[b][:, ti * 128 : (ti + 1) * 128],
                                wp_sb[:, fb * TB : (fb + 1) * TB],
                                start=True,
                                stop=True,
                            )
                            ysb = ypool.tile([128, TB], F32, name="ysb", tag="ysb")
                            nc.vector.tensor_copy(out=ysb[:], in_=ps[:])
                            nc.sync.dma_start(
                                y[b * T + ti * 128 : b * T + (ti + 1) * 128,
                                  fb * TB : (fb + 1) * TB],
                                ysb[:],
                            )

                pending_proj.append(proj_step)
        emit_pending_proj()
    nc.compile()
    return nc


def make_in_maps(x, w_attn, w_proj):
    """Host-side sharding into the per-core layouts."""
    x = np.asarray(x, dtype=np.float32)
    w_attn = np.asarray(w_attn, dtype=np.float32)
    w_proj = np.asarray(w_proj, dtype=np.float32)

    xT = np.ascontiguousarray(x.reshape(BT, C).T.astype(np.float16))
    wpT_full = np.ascontiguousarray(w_proj.T.astype(np.float16))

    in_maps = []
    for c in range(NCORES):
        rows = []
        for sec in range(3):                                # q, k, v
            for h in (HPC * c, HPC * c + 1):
                rows.extend(range(sec * C + h * D, sec * C + (h + 1) * D))
        wqkvT = np.ascontiguousarray(w_attn[rows, :].T.astype(np.float16))
        wpT = np.ascontiguousarray(
            wpT_full[c * HPC * D : (c + 1) * HPC * D, :]    # [128, 1024]
        )
        consts = np.stack(
            [
                np.eye(128, dtype=np.float16),
                np.tril(np.ones((128, 128), np.float16)).T,  # keep kt <= qt
            ]
        )
        in_maps.append({"xT": xT, "wqkvT": wqkvT, "wpT": wpT, "consts": consts})
    return in_maps


_PROGRAM = None


def _program():
    global _PROGRAM
    if _PROGRAM is None:
        _PROGRAM = build_program()
    return _PROGRAM


def kernel(x, w_attn, w_proj):
    from concourse.bass_utils import run_bass_kernel_spmd

    res = run_bass_kernel_spmd(
        _program(), make_in_maps(x, w_attn, w_proj), list(range(NCORES))
    )
    out = res.results[0]["y"].astype(np.float32, copy=True)
    for i in range(1, NCORES):
        out += res.results[i]["y"]
    return out.reshape(B, T, C)


# revision 27
# speedup vs baseline: 1.2380x; 1.0277x over previous
"""Causal self-attention on 8 Trainium2 NeuronCores.

Sharding: 2 heads per core (tensor parallel).  The host pre-transposes the
activations/weights into the layouts the PE array wants, each core computes
QKV -> causal attention -> its partial of the output projection for its two
heads, and the host sums the 8 partial projections (row-parallel linear).

Per-core device program (SPMD, different data per core):
  xT    [1024, 4096]  x transposed, rows=embed c, cols=token t (t = b*2048+tt)
  wqkvT [1024, 384]   w_attn rows for this core's heads, transposed.
                      f = [q_h0 d0..63 | q_h1 | k_h0 | k_h1 | v_h0 | v_h1]
  wpT   [128, 1024]   w_proj columns for this core's channels, transposed
  y     [4096, 1024]  partial output (sum over cores = final)

Dataflow (everything "transposed" so the PE contraction dim is the partition
dim with no on-device transposes of activations):
  qkvT[f, t]   = wqkvT_tile.T @ xT_tile            (accumulate over 8 c-tiles)
  S^T[kt, qt]  = kT_tile.T @ qT_block              (K = head dim 64)
  P^T          = exp(S^T / 32)                     (ACT; no max subtraction --
                                                    scores are O(1), exp safe)
  causal mask  = multiply diagonal 128x128 block by 0/1 lower-tri tile
  outT[65,qt] += [V | ones].T @ P^T                (row 64 = softmax sums)
  attnT        = outT[0:64] * (1 / outT[64])       (broadcast along partitions)
  y[t, f]      = attnT_tile.T @ wpT                (partial; host sums cores)

All matmuls run as float32r (fp32 bitcast): 1 PE cycle/row when the moving
free dim is >= 256 -- full bf16-class speed with ~fp22 mantissa precision.
"""

import numpy as np

B, T, C = 2, 2048, 1024
H, D = 16, 64
NCORES = 8
HPC = H // NCORES          # heads per core = 2
BT = B * T                 # 4096 tokens total
TB = 512                   # token block (matmul moving free dim)
CK = C // 128              # 8 contraction tiles for the projections
NTB = BT // TB             # 8 token blocks
NQB = T // TB              # 4 q blocks per batch
NKT = T // 128             # 16 kt tiles per batch
SCALE = 1.0 / 32.0         # 1 / sqrt(C)


def build_program():
    """Build the single-core Bass program (same program runs on all 8 cores)."""
    from contextlib import ExitStack

    import concourse.mybir as mybir
    import concourse.tile as tile
    from concourse import bacc, library_config

    dt = mybir.dt
    F32 = dt.float32
    F32R = dt.float32r
    F16 = dt.float16

    nc = bacc.Bacc("TRN2")
    xT = nc.dram_tensor("xT", [C, BT], F16, kind="ExternalInput").ap()
    wqkvT = nc.dram_tensor("wqkvT", [C, 3 * HPC * D], F16, kind="ExternalInput").ap()
    wpT = nc.dram_tensor("wpT", [HPC * D, C], F16, kind="ExternalInput").ap()
    # consts[0] = 128x128 identity, consts[1] = causal keep-mask
    # (mask[kt, qt] = 1.0 where kt <= qt)
    consts = nc.dram_tensor("consts", [2, 128, 128], F16, kind="ExternalInput").ap()
    y = nc.dram_tensor("y", [BT, C], F32, kind="ExternalOutput").ap()

    with ExitStack() as ctx:
        tc = ctx.enter_context(tile.TileContext(nc))
        const = ctx.enter_context(tc.tile_pool(name="const", bufs=1))
        xpool = ctx.enter_context(tc.tile_pool(name="xload", bufs=16))
        ppool = ctx.enter_context(tc.tile_pool(name="pexp", bufs=6))
        npool = ctx.enter_context(tc.tile_pool(name="norm", bufs=6))
        pvpool = ctx.enter_context(tc.tile_pool(name="pvs", bufs=4))
        ypool = ctx.enter_context(tc.tile_pool(name="yout", bufs=4))
        psA = ctx.enter_context(tc.tile_pool(name="psA", bufs=2, space="PSUM"))
        psPV = ctx.enter_context(tc.tile_pool(name="psPV", bufs=2, space="PSUM"))

        # ---------- constants / persistent SBUF ----------
        w_sb = const.tile([128, CK, 3 * HPC * D], F16, name="w_sb")
        wqkvT_t = wqkvT.rearrange("(a p) f -> p a f", p=128)
        for ci in range(CK):
            nc.sync.dma_start(w_sb[:, ci, :], wqkvT_t[:, ci, :])
        wp_sb = const.tile([128, C], F16, name="wp_sb")
        nc.sync.dma_start(wp_sb[:], wpT)

        ident = const.tile([128, 128], F16, name="ident")
        nc.sync.dma_start(ident[:], consts[0])
        trimask2 = const.tile([128, HPC, 128], F16, name="trimask2")
        for _h in range(HPC):
            nc.sync.dma_start(trimask2[:, _h, :], consts[1])
        # partition_broadcast lives in the "attn" GPSIMD library; same-engine
        # FIFO order guarantees this lands before the broadcasts.
        nc.gpsimd.load_library(library_config.attn)

        # Per-batch transposed activations, heads packed on partitions
        # (h0 -> partitions 0:64, h1 -> 64:128).
        qT = [const.tile([128, T], F16, name=f"qT{b}") for b in range(B)]
        kT = [const.tile([128, T], F16, name=f"kT{b}") for b in range(B)]
        vT = [const.tile([128, T], F16, name=f"vT{b}") for b in range(B)]
        attnT = [const.tile([128, T], F16, name=f"attnT{b}") for b in range(B)]

        # [V | ones] stationary tiles for PV: V1[:, b, h, kti, 0:64] = V natural
        # [kt, d]; column 64 = 1.0 so PV row 64 accumulates the softmax sums.
        V1 = const.tile([128, B, HPC, NKT, 65], F16, name="V1")
        nc.vector.memset(V1[:, :, :, :, 64:65], 1.0)

        # ---------- phase 1: QKV projection ----------
        dest = {0: qT, 1: kT, 2: vT}
        for tb in range(NTB):
            b, tcol = divmod(tb, NTB // B)
            xts = []
            for ci in range(CK):
                xt = xpool.tile([128, TB], F16, name="xt", tag="xt")
                nc.sync.dma_start(
                    xt[:],
                    xT[ci * 128 : (ci + 1) * 128, tb * TB : (tb + 1) * TB],
                )
                xts.append(xt)
            qtags = ["psA", "psPV0", "psPV1"]
            pss = [
                psA.tile([128, TB], F32, name="qkv_ps", tag="psA")
                if fi == 0
                else psPV.tile([128, TB], F32, name=f"qkv_ps{fi}", tag=qtags[fi])
                for fi in range(3)
            ]
            for ci in range(CK):
                for fi in range(3):
                    nc.tensor.matmul(
                        pss[fi][:],
                        w_sb[:, ci, fi * 128 : (fi + 1) * 128],
                        xts[ci][:],
                        start=(ci == 0),
                        stop=(ci == CK - 1),
                    )
            for fi in range(3):
                nc.scalar.copy(
                    out=dest[fi][b][:, tcol * TB : (tcol + 1) * TB], in_=pss[fi][:]
                )

            # As soon as a batch's vT is complete, build its V-natural tiles
            # (PE transpose of 64-row slices through the identity).
            if tcol == NTB // B - 1:
                for h in range(HPC):
                    hp = slice(h * 64, (h + 1) * 64)
                    for kti in range(NKT):
                        tr = psA.tile([128, 64], F16, name="vtr", tag="psA")
                        nc.tensor.transpose(
                            tr[:], vT[b][hp, kti * 128 : (kti + 1) * 128], ident[hp, hp]
                        )
                        nc.vector.tensor_copy(out=V1[:, b, h, kti, 0:64], in_=tr[:])

        # ---------- phase 2: causal attention ----------
        # Both heads interleaved per (b, qb) and PV software-pipelined one kt
        # tile behind the scores so the PE never stalls on the ACT exp.
        # Unnormalized [PV | sums] results are copied to SBUF (freeing PSUM)
        # and all 16 sum-rows are collected so one batched reciprocal covers
        # the whole kernel (a [1, N] DVE reciprocal is ~3.4 us — single lane).
        pending_proj = []

        def emit_pending_proj(half=False):
            n = max(1, len(pending_proj) // 2) if half else len(pending_proj)
            for _ in range(min(n, len(pending_proj))):
                pending_proj.pop(0)()

        for b in range(B):
            for qb in range(NQB):
                nkt = 4 * qb + 4
                pv = [
                    psPV.tile([65, TB], F32, name=f"pv_ps{h}", tag=f"psPV{h}")
                    for h in range(HPC)
                ]
                stages = []  # deferred PV matmuls, one kti behind the scores

                def flush(n=None):
                    while stages and (n is None or len(stages) > n):
                        stages.pop(0)()

                for kti in range(nkt):
                    qs = max(0, kti * 128 - qb * TB)  # local col start
                    N = TB - qs
                    # both heads' scores in one 2-bank PSUM tile -> one exp
                    sps = psA.tile([128, HPC, TB], F32, name="s_ps", tag="psA")
                    for h in range(HPC):
                        hp = slice(h * 64, (h + 1) * 64)
                        nc.tensor.matmul(
                            sps[:, h, 0:N],
                            kT[b][hp, kti * 128 : (kti + 1) * 128],
                            qT[b][hp, qb * TB + qs : (qb + 1) * TB],
                            start=True,
                            stop=True,
                        )
                    P = ppool.tile([128, HPC, TB], F16, name="Pt", tag="P")
                    nc.scalar.activation(
                        P[:, :, 0:N],
                        sps[:, :, 0:N],
                        mybir.ActivationFunctionType.Exp,
                        scale=SCALE,
                    )
                    if kti * 128 >= qb * TB:
                        # diagonal tile: first 128 cols of each head hold the
                        # triangle; one DVE mult covers both heads
                        nc.vector.tensor_mul(
                            P[:, :, 0:128], P[:, :, 0:128], trimask2[:]
                        )

                    def pv_step(kti=kti, qs=qs, N=N, P=P):
                        for h in range(HPC):
                            nc.tensor.matmul(
                                pv[h][:, qs:TB],
                                V1[:, b, h, kti, :],
                                P[:, h, 0:N],
                                start=(kti == 0),
                                stop=(kti == nkt - 1),
                            )

                    stages.append(pv_step)
                    flush(1)
                    if kti in (1, 3):
                        emit_pending_proj(half=(kti == 1))
                flush()

                # normalize this q-block inline (reciprocal_approx_fast is
                # ~18-bit accurate, plenty above the fp32r noise floor), then
                # emit its projection: the proj matmuls are exp-independent
                # PE work that fills the next q-block's ACT stalls.
                for h in range(HPC):
                    hp = slice(h * 64, (h + 1) * 64)
                    pvt = pvpool.tile([65, TB], F32, name="pvt", tag="pvt")
                    nc.vector.tensor_copy(out=pvt[:], in_=pv[h][:])
                    # custom-DVE ops require partition-0 sources on HW; plain
                    # copies handle the 64->0 partition shift fine.
                    s0 = npool.tile([1, TB], F32, name="s0", tag="s0")
                    nc.vector.tensor_copy(out=s0[:], in_=pvt[64:65, :])
                    rt = npool.tile([1, TB], F32, name="rt", tag="rt")
                    nc.vector.reciprocal_approx_fast(rt[:], s0[:])
                    bc = npool.tile([64, TB], F32, name="bc", tag="bc")
                    nc.gpsimd.partition_broadcast(bc[:], rt[:])
                    nc.vector.tensor_mul(
                        attnT[b][hp, qb * TB : (qb + 1) * TB], pvt[0:64, :], bc[:]
                    )
                def proj_step(b=b, qb=qb):
                    for ti in range(4 * qb, 4 * qb + 4):
                        for fb in range(C // TB):
                            ps = psA.tile([128, TB], F32, name="y_ps", tag="psA")
                            nc.tensor.matmul(
                                ps[:],
                                attnT[b][:, ti * 128 : (ti + 1) * 128],
                                wp_sb[:, fb * TB : (fb + 1) * TB],
                                start=True,
                                stop=True,
                            )
                            ysb = ypool.tile([128, TB], F32, name="ysb", tag="ysb")
                            nc.vector.tensor_copy(out=ysb[:], in_=ps[:])
                            nc.sync.dma_start(
                                y[b * T + ti * 128 : b * T + (ti + 1) * 128,
                                  fb * TB : (fb + 1) * TB],
                                ysb[:],
                            )

                pending_proj.append(proj_step)
        emit_pending_proj()
    nc.compile()
    return nc


def make_in_maps(x, w_attn, w_proj):
    """Host-side sharding into the per-core layouts."""
    x = np.asarray(x, dtype=np.float32)
    w_attn = np.asarray(w_attn, dtype=np.float32)
    w_proj = np.asarray(w_proj, dtype=np.float32)

    xT = np.ascontiguousarray(x.reshape(BT, C).T.astype(np.float16))
    wpT_full = np.ascontiguousarray(w_proj.T.astype(np.float16))

    in_maps = []
    for c in range(NCORES):
        rows = []
        for sec in range(3):                                # q, k, v
            for h in (HPC * c, HPC * c + 1):
                rows.extend(range(sec * C + h * D, sec * C + (h + 1) * D))
        wqkvT = np.ascontiguousarray(w_attn[rows, :].T.astype(np.float16))
        wpT = np.ascontiguousarray(
            wpT_full[c * HPC * D : (c + 1) * HPC * D, :]    # [128, 1024]
        )
        consts = np.stack(
            [
                np.eye(128, dtype=np.float16),
                np.tril(np.ones((128, 128), np.float16)).T,  # keep kt <= qt
            ]
        )
        in_maps.append({"xT": xT, "wqkvT": wqkvT, "wpT": wpT, "consts": consts})
    return in_maps


_PROGRAM = None


def _program():
    global _PROGRAM
    if _PROGRAM is None:
        _PROGRAM = build_program()
    return _PROGRAM


def kernel(x, w_attn, w_proj):
    from concourse.bass_utils import run_bass_kernel_spmd

    res = run_bass_kernel_spmd(
        _program(), make_in_maps(x, w_attn, w_proj), list(range(NCORES))
    )
    out = res.results[0]["y"].astype(np.float32, copy=True)
    for i in range(1, NCORES):
        out += res.results[i]["y"]
    return out.reshape(B, T, C)


# revision 29
# speedup vs baseline: 1.2727x; 1.0280x over previous
"""Causal self-attention on 8 Trainium2 NeuronCores.

Sharding: 2 heads per core (tensor parallel).  The host pre-transposes the
activations/weights into the layouts the PE array wants, each core computes
QKV -> causal attention -> its partial of the output projection for its two
heads, and the host sums the 8 partial projections (row-parallel linear).

Per-core device program (SPMD, different data per core):
  xT    [1024, 4096]  x transposed, rows=embed c, cols=token t (t = b*2048+tt)
  wqkvT [1024, 384]   w_attn rows for this core's heads, transposed.
                      f = [q_h0 d0..63 | q_h1 | k_h0 | k_h1 | v_h0 | v_h1]
  wpT   [128, 1024]   w_proj columns for this core's channels, transposed
  y     [4096, 1024]  partial output (sum over cores = final)

Dataflow (everything "transposed" so the PE contraction dim is the partition
dim with no on-device transposes of activations):
  qkvT[f, t]   = wqkvT_tile.T @ xT_tile            (accumulate over 8 c-tiles)
  S^T[kt, qt]  = kT_tile.T @ qT_block              (K = head dim 64)
  P^T          = exp(S^T / 32)                     (ACT; no max subtraction --
                                                    scores are O(1), exp safe)
  causal mask  = multiply diagonal 128x128 block by 0/1 lower-tri tile
  outT[65,qt] += [V | ones].T @ P^T                (row 64 = softmax sums)
  attnT        = outT[0:64] * (1 / outT[64])       (broadcast along partitions)
  y[t, f]      = attnT_tile.T @ wpT                (partial; host sums cores)

All matmul operands are float16 (host-converted): 2-byte operands stream at
1 PE cycle/row (fp32/fp32r is SBUF-read-bandwidth-bound at ~2 cycles/row) and
halve the HBM traffic; accumulation stays fp32 in PSUM, the final projection
output and the host-side cross-core sum are fp32.  Measured end-to-end
relative error vs the fp32 reference: ~5e-4.

Measured on 8 axon trn2 cores: ~272 us HW exec (from a 395 us fp32r
baseline).  Key scheduling facts baked in: Tile's PE order is static, so the
projection of q-block i is *emitted* inside q-block i+1's score loop to fill
the PE stalls left by the ACT exp; PV matmuls trail scores by one kt tile;
softmax uses no max-subtraction (scores are O(1)); normalization is
reciprocal_approx_fast + gpsimd partition_broadcast (source must sit at
partition 0 on HW).
"""

import numpy as np

B, T, C = 2, 2048, 1024
H, D = 16, 64
NCORES = 8
HPC = H // NCORES          # heads per core = 2
BT = B * T                 # 4096 tokens total
TB = 512                   # token block (matmul moving free dim)
CK = C // 128              # 8 contraction tiles for the projections
NTB = BT // TB             # 8 token blocks
NQB = T // TB              # 4 q blocks per batch
NKT = T // 128             # 16 kt tiles per batch
SCALE = 1.0 / 32.0         # 1 / sqrt(C)


def build_program():
    """Build the single-core Bass program (same program runs on all 8 cores)."""
    from contextlib import ExitStack

    import concourse.mybir as mybir
    import concourse.tile as tile
    from concourse import bacc, library_config

    dt = mybir.dt
    F32 = dt.float32
    F32R = dt.float32r
    F16 = dt.float16

    nc = bacc.Bacc("TRN2")
    xT = nc.dram_tensor("xT", [C, BT], F16, kind="ExternalInput").ap()
    wqkvT = nc.dram_tensor("wqkvT", [C, 3 * HPC * D], F16, kind="ExternalInput").ap()
    wpT = nc.dram_tensor("wpT", [HPC * D, C], F16, kind="ExternalInput").ap()
    # consts[0] = 128x128 identity, consts[1] = causal keep-mask
    # (mask[kt, qt] = 1.0 where kt <= qt)
    consts = nc.dram_tensor("consts", [2, 128, 128], F16, kind="ExternalInput").ap()
    y = nc.dram_tensor("y", [BT, C], F32, kind="ExternalOutput").ap()

    with ExitStack() as ctx:
        tc = ctx.enter_context(tile.TileContext(nc))
        const = ctx.enter_context(tc.tile_pool(name="const", bufs=1))
        xpool = ctx.enter_context(tc.tile_pool(name="xload", bufs=16))
        ppool = ctx.enter_context(tc.tile_pool(name="pexp", bufs=6))
        npool = ctx.enter_context(tc.tile_pool(name="norm", bufs=6))
        pvpool = ctx.enter_context(tc.tile_pool(name="pvs", bufs=4))
        ypool = ctx.enter_context(tc.tile_pool(name="yout", bufs=4))
        psA = ctx.enter_context(tc.tile_pool(name="psA", bufs=2, space="PSUM"))
        psPV = ctx.enter_context(tc.tile_pool(name="psPV", bufs=2, space="PSUM"))

        # ---------- constants / persistent SBUF ----------
        w_sb = const.tile([128, CK, 3 * HPC * D], F16, name="w_sb")
        wqkvT_t = wqkvT.rearrange("(a p) f -> p a f", p=128)
        for ci in range(CK):
            nc.sync.dma_start(w_sb[:, ci, :], wqkvT_t[:, ci, :])
        wp_sb = const.tile([128, C], F16, name="wp_sb")
        nc.sync.dma_start(wp_sb[:], wpT)

        ident = const.tile([128, 128], F16, name="ident")
        nc.sync.dma_start(ident[:], consts[0])
        trimask2 = const.tile([128, HPC, 128], F16, name="trimask2")
        for _h in range(HPC):
            nc.sync.dma_start(trimask2[:, _h, :], consts[1])
        # partition_broadcast lives in the "attn" GPSIMD library; same-engine
        # FIFO order guarantees this lands before the broadcasts.
        nc.gpsimd.load_library(library_config.attn)

        # Per-batch transposed activations, heads packed on partitions
        # (h0 -> partitions 0:64, h1 -> 64:128).
        qT = [const.tile([128, T], F16, name=f"qT{b}") for b in range(B)]
        kT = [const.tile([128, T], F16, name=f"kT{b}") for b in range(B)]
        vT = [const.tile([128, T], F16, name=f"vT{b}") for b in range(B)]
        attnT = [const.tile([128, T], F16, name=f"attnT{b}") for b in range(B)]

        # [V | ones] stationary tiles for PV: V1[:, b, h, kti, 0:64] = V natural
        # [kt, d]; column 64 = 1.0 so PV row 64 accumulates the softmax sums.
        V1 = const.tile([128, B, HPC, NKT, 65], F16, name="V1")
        nc.vector.memset(V1[:, :, :, :, 64:65], 1.0)

        # ---------- phase 1: QKV projection ----------
        dest = {0: qT, 1: kT, 2: vT}

        def qkv_block(tb):
            b, tcol = divmod(tb, NTB // B)
            xts = []
            for ci in range(CK):
                xt = xpool.tile([128, TB], F16, name="xt", tag="xt")
                nc.sync.dma_start(
                    xt[:],
                    xT[ci * 128 : (ci + 1) * 128, tb * TB : (tb + 1) * TB],
                )
                xts.append(xt)
            qtags = ["psA", "psPV0", "psPV1"]
            pss = [
                psA.tile([128, TB], F32, name="qkv_ps", tag="psA")
                if fi == 0
                else psPV.tile([128, TB], F32, name=f"qkv_ps{fi}", tag=qtags[fi])
                for fi in range(3)
            ]
            for ci in range(CK):
                for fi in range(3):
                    nc.tensor.matmul(
                        pss[fi][:],
                        w_sb[:, ci, fi * 128 : (fi + 1) * 128],
                        xts[ci][:],
                        start=(ci == 0),
                        stop=(ci == CK - 1),
                    )
            for fi in range(3):
                nc.vector.tensor_copy(
                    out=dest[fi][b][:, tcol * TB : (tcol + 1) * TB], in_=pss[fi][:]
                )

            # As soon as a batch's vT is complete, build its V-natural tiles
            # (PE transpose of 64-row slices through the identity).
            if tcol == NTB // B - 1:
                for h in range(HPC):
                    hp = slice(h * 64, (h + 1) * 64)
                    for kti in range(NKT):
                        tr = psA.tile([128, 64], F16, name="vtr", tag="psA")
                        nc.tensor.transpose(
                            tr[:], vT[b][hp, kti * 128 : (kti + 1) * 128], ident[hp, hp]
                        )
                        nc.vector.tensor_copy(out=V1[:, b, h, kti, 0:64], in_=tr[:])

        for tb in range(NTB // B):
            qkv_block(tb)

        # ---------- phase 2: causal attention ----------
        # Both heads interleaved per (b, qb) and PV software-pipelined one kt
        # tile behind the scores so the PE never stalls on the ACT exp.
        # Unnormalized [PV | sums] results are copied to SBUF (freeing PSUM)
        # and all 16 sum-rows are collected so one batched reciprocal covers
        # the whole kernel (a [1, N] DVE reciprocal is ~3.4 us — single lane).
        pending = []
        for tb in range(NTB // B, NTB):
            pending.append(lambda tb=tb: qkv_block(tb))

        def emit_pending(n=None):
            cnt = len(pending) if n is None else min(n, len(pending))
            for _ in range(cnt):
                pending.pop(0)()

        for b in range(B):
            # batch 1's attention consumes batch-1 QKV/V1: those must be
            # emitted (Tile dep-tracking follows emission order) before it.
            if b == 1:
                emit_pending()
            for qb in range(NQB):
                nkt = 4 * qb + 4
                pv = [
                    psPV.tile([65, TB], F32, name=f"pv_ps{h}", tag=f"psPV{h}")
                    for h in range(HPC)
                ]
                stages = []  # deferred PV matmuls, one kti behind the scores

                def flush(n=None):
                    while stages and (n is None or len(stages) > n):
                        stages.pop(0)()

                for kti in range(nkt):
                    qs = max(0, kti * 128 - qb * TB)  # local col start
                    N = TB - qs
                    # both heads' scores in one 2-bank PSUM tile -> one exp
                    sps = psA.tile([128, HPC, TB], F32, name="s_ps", tag="psA")
                    for h in range(HPC):
                        hp = slice(h * 64, (h + 1) * 64)
                        nc.tensor.matmul(
                            sps[:, h, 0:N],
                            kT[b][hp, kti * 128 : (kti + 1) * 128],
                            qT[b][hp, qb * TB + qs : (qb + 1) * TB],
                            start=True,
                            stop=True,
                        )
                    P = ppool.tile([128, HPC, TB], F16, name="Pt", tag="P")
                    nc.scalar.activation(
                        P[:, :, 0:N],
                        sps[:, :, 0:N],
                        mybir.ActivationFunctionType.Exp,
                        scale=SCALE,
                    )
                    if kti * 128 >= qb * TB:
                        # diagonal tile: first 128 cols of each head hold the
                        # triangle; one DVE mult covers both heads
                        nc.vector.tensor_mul(
                            P[:, :, 0:128], P[:, :, 0:128], trimask2[:]
                        )

                    def pv_step(kti=kti, qs=qs, N=N, P=P):
                        for h in range(HPC):
                            nc.tensor.matmul(
                                pv[h][:, qs:TB],
                                V1[:, b, h, kti, :],
                                P[:, h, 0:N],
                                start=(kti == 0),
                                stop=(kti == nkt - 1),
                            )

                    stages.append(pv_step)
                    flush(1)
                    if kti in (1, 3):
                        emit_pending(1)
                flush()

                # normalize this q-block inline (reciprocal_approx_fast is
                # ~18-bit accurate, plenty above the fp32r noise floor), then
                # emit its projection: the proj matmuls are exp-independent
                # PE work that fills the next q-block's ACT stalls.
                for h in range(HPC):
                    hp = slice(h * 64, (h + 1) * 64)
                    pvt = pvpool.tile([65, TB], F32, name="pvt", tag="pvt")
                    nc.vector.tensor_copy(out=pvt[:], in_=pv[h][:])
                    # custom-DVE ops require partition-0 sources on HW; plain
                    # copies handle the 64->0 partition shift fine.
                    s0 = npool.tile([1, TB], F32, name="s0", tag="s0")
                    nc.vector.tensor_copy(out=s0[:], in_=pvt[64:65, :])
                    rt = npool.tile([1, TB], F32, name="rt", tag="rt")
                    nc.vector.reciprocal_approx_fast(rt[:], s0[:])
                    bc = npool.tile([64, TB], F32, name="bc", tag="bc")
                    nc.gpsimd.partition_broadcast(bc[:], rt[:])
                    nc.vector.tensor_mul(
                        attnT[b][hp, qb * TB : (qb + 1) * TB], pvt[0:64, :], bc[:]
                    )
                def proj_step(b=b, qb=qb):
                    for ti in range(4 * qb, 4 * qb + 4):
                        for fb in range(C // TB):
                            ps = psA.tile([128, TB], F32, name="y_ps", tag="psA")
                            nc.tensor.matmul(
                                ps[:],
                                attnT[b][:, ti * 128 : (ti + 1) * 128],
                                wp_sb[:, fb * TB : (fb + 1) * TB],
                                start=True,
                                stop=True,
                            )
                            ysb = ypool.tile([128, TB], F32, name="ysb", tag="ysb")
                            nc.vector.tensor_copy(out=ysb[:], in_=ps[:])
                            nc.sync.dma_start(
                                y[b * T + ti * 128 : b * T + (ti + 1) * 128,
                                  fb * TB : (fb + 1) * TB],
                                ysb[:],
                            )

                pending.append(proj_step)
        emit_pending()
    nc.compile()
    return nc


def make_in_maps(x, w_attn, w_proj):
    """Host-side sharding into the per-core layouts."""
    x = np.asarray(x, dtype=np.float32)
    w_attn = np.asarray(w_attn, dtype=np.float32)
    w_proj = np.asarray(w_proj, dtype=np.float32)

    xT = np.ascontiguousarray(x.reshape(BT, C).T.astype(np.float16))
    wpT_full = np.ascontiguousarray(w_proj.T.astype(np.float16))

    in_maps = []
    for c in range(NCORES):
        rows = []
        for sec in range(3):                                # q, k, v
            for h in (HPC * c, HPC * c + 1):
                rows.extend(range(sec * C + h * D, sec * C + (h + 1) * D))
        wqkvT = np.ascontiguousarray(w_attn[rows, :].T.astype(np.float16))
        wpT = np.ascontiguousarray(
            wpT_full[c * HPC * D : (c + 1) * HPC * D, :]    # [128, 1024]
        )
        consts = np.stack(
            [
                np.eye(128, dtype=np.float16),
                np.tril(np.ones((128, 128), np.float16)).T,  # keep kt <= qt
            ]
        )
        in_maps.append({"xT": xT, "wqkvT": wqkvT, "wpT": wpT, "consts": consts})
    return in_maps


_PROGRAM = None


def _program():
    global _PROGRAM
    if _PROGRAM is None:
        _PROGRAM = build_program()
    return _PROGRAM


def kernel(x, w_attn, w_proj):
    from concourse.bass_utils import run_bass_kernel_spmd

    res = run_bass_kernel_spmd(
        _program(), make_in_maps(x, w_attn, w_proj), list(range(NCORES))
    )
    out = res.results[0]["y"].astype(np.float32, copy=True)
    for i in range(1, NCORES):
        out += res.results[i]["y"]
    return out.reshape(B, T, C)


# revision 30
# speedup vs baseline: 1.3366x; 1.0502x over previous
"""Causal self-attention on 8 Trainium2 NeuronCores.

Sharding: 2 heads per core (tensor parallel).  The host pre-transposes the
activations/weights into the layouts the PE array wants, each core computes
QKV -> causal attention -> its partial of the output projection for its two
heads, and the host sums the 8 partial projections (row-parallel linear).

Per-core device program (SPMD, different data per core):
  xT    [1024, 4096]  x transposed, rows=embed c, cols=token t (t = b*2048+tt)
  wqkvT [1024, 384]   w_attn rows for this core's heads, transposed.
                      f = [q_h0 d0..63 | q_h1 | k_h0 | k_h1 | v_h0 | v_h1]
  wpT   [128, 1024]   w_proj columns for this core's channels, transposed
  y     [4096, 1024]  partial output (sum over cores = final)

Dataflow (everything "transposed" so the PE contraction dim is the partition
dim with no on-device transposes of activations):
  qkvT[f, t]   = wqkvT_tile.T @ xT_tile            (accumulate over 8 c-tiles)
  S^T[kt, qt]  = kT_tile.T @ qT_block              (K = head dim 64)
  P^T          = exp(S^T / 32)                     (ACT; no max subtraction --
                                                    scores are O(1), exp safe)
  causal mask  = multiply diagonal 128x128 block by 0/1 lower-tri tile
  outT[65,qt] += [V | ones].T @ P^T                (row 64 = softmax sums)
  attnT        = outT[0:64] * (1 / outT[64])       (broadcast along partitions)
  y[t, f]      = attnT_tile.T @ wpT                (partial; host sums cores)

All matmul operands are float16 (host-converted): 2-byte operands stream at
1 PE cycle/row (fp32/fp32r is SBUF-read-bandwidth-bound at ~2 cycles/row) and
halve the HBM traffic; accumulation stays fp32 in PSUM, the final projection
output and the host-side cross-core sum are fp32.  Measured end-to-end
relative error vs the fp32 reference: ~5e-4.

Measured on 8 axon trn2 cores: ~272 us HW exec (from a 395 us fp32r
baseline).  Key scheduling facts baked in: Tile's PE order is static, so the
projection of q-block i is *emitted* inside q-block i+1's score loop to fill
the PE stalls left by the ACT exp; PV matmuls trail scores by one kt tile;
softmax uses no max-subtraction (scores are O(1)); normalization is
reciprocal_approx_fast + gpsimd partition_broadcast (source must sit at
partition 0 on HW).
"""

import numpy as np

B, T, C = 2, 2048, 1024
H, D = 16, 64
NCORES = 8
HPC = H // NCORES          # heads per core = 2
BT = B * T                 # 4096 tokens total
TB = 512                   # token block (matmul moving free dim)
CK = C // 128              # 8 contraction tiles for the projections
NTB = BT // TB             # 8 token blocks
NQB = T // TB              # 4 q blocks per batch
NKT = T // 128             # 16 kt tiles per batch
SCALE = 1.0 / 32.0         # 1 / sqrt(C)


def build_program():
    """Build the single-core Bass program (same program runs on all 8 cores)."""
    from contextlib import ExitStack

    import concourse.mybir as mybir
    import concourse.tile as tile
    from concourse import bacc, library_config

    dt = mybir.dt
    F32 = dt.float32
    F32R = dt.float32r
    F16 = dt.float16

    nc = bacc.Bacc("TRN2")
    xT = nc.dram_tensor("xT", [C, BT], F16, kind="ExternalInput").ap()
    wqkvT = nc.dram_tensor("wqkvT", [C, 3 * HPC * D], F16, kind="ExternalInput").ap()
    wpT = nc.dram_tensor("wpT", [HPC * D, C], F16, kind="ExternalInput").ap()
    # consts[0] = 128x128 identity, consts[1] = causal keep-mask
    # (mask[kt, qt] = 1.0 where kt <= qt)
    consts = nc.dram_tensor("consts", [2, 128, 128], F16, kind="ExternalInput").ap()
    y = nc.dram_tensor("y", [BT, C], F32, kind="ExternalOutput").ap()

    with ExitStack() as ctx:
        tc = ctx.enter_context(tile.TileContext(nc))
        const = ctx.enter_context(tc.tile_pool(name="const", bufs=1))
        xpool = ctx.enter_context(tc.tile_pool(name="xload", bufs=16))
        ppool = ctx.enter_context(tc.tile_pool(name="pexp", bufs=6))
        npool = ctx.enter_context(tc.tile_pool(name="norm", bufs=6))
        pvpool = ctx.enter_context(tc.tile_pool(name="pvs", bufs=4))
        ypool = ctx.enter_context(tc.tile_pool(name="yout", bufs=4))
        psA = ctx.enter_context(tc.tile_pool(name="psA", bufs=2, space="PSUM"))
        psPV = ctx.enter_context(tc.tile_pool(name="psPV", bufs=2, space="PSUM"))

        # ---------- constants / persistent SBUF ----------
        w_sb = const.tile([128, CK, 3 * HPC * D], F16, name="w_sb")
        wqkvT_t = wqkvT.rearrange("(a p) f -> p a f", p=128)
        for ci in range(CK):
            nc.sync.dma_start(w_sb[:, ci, :], wqkvT_t[:, ci, :])
        wp_sb = const.tile([128, C], F16, name="wp_sb")
        nc.sync.dma_start(wp_sb[:], wpT)

        ident = const.tile([128, 128], F16, name="ident")
        nc.sync.dma_start(ident[:], consts[0])
        trimask2 = const.tile([128, HPC, 128], F16, name="trimask2")
        for _h in range(HPC):
            nc.sync.dma_start(trimask2[:, _h, :], consts[1])
        # partition_broadcast lives in the "attn" GPSIMD library; same-engine
        # FIFO order guarantees this lands before the broadcasts.
        nc.gpsimd.load_library(library_config.attn)

        # Per-batch transposed activations, heads packed on partitions
        # (h0 -> partitions 0:64, h1 -> 64:128).
        qT = [const.tile([128, T], F16, name=f"qT{b}") for b in range(B)]
        kT = [const.tile([128, T], F16, name=f"kT{b}") for b in range(B)]
        vT = [const.tile([128, T], F16, name=f"vT{b}") for b in range(B)]
        attnT = [const.tile([128, T], F16, name=f"attnT{b}") for b in range(B)]

        # [V | ones] stationary tiles for PV: V1[:, b, h, kti, 0:64] = V natural
        # [kt, d]; column 64 = 1.0 so PV row 64 accumulates the softmax sums.
        V1 = const.tile([128, B, HPC, NKT, 65], F16, name="V1")
        nc.vector.memset(V1[:, :, :, :, 64:65], 1.0)

        # ---------- phase 1: QKV projection ----------
        dest = {0: qT, 1: kT, 2: vT}

        def qkv_block(tb):
            b, tcol = divmod(tb, NTB // B)
            xts = []
            for ci in range(CK):
                xt = xpool.tile([128, TB], F16, name="xt", tag="xt")
                nc.sync.dma_start(
                    xt[:],
                    xT[ci * 128 : (ci + 1) * 128, tb * TB : (tb + 1) * TB],
                )
                xts.append(xt)
            qtags = ["psA", "psPV0", "psPV1"]
            pss = [
                psA.tile([128, TB], F32, name="qkv_ps", tag="psA")
                if fi == 0
                else psPV.tile([128, TB], F32, name=f"qkv_ps{fi}", tag=qtags[fi])
                for fi in range(3)
            ]
            for ci in range(CK):
                for fi in range(3):
                    nc.tensor.matmul(
                        pss[fi][:],
                        w_sb[:, ci, fi * 128 : (fi + 1) * 128],
                        xts[ci][:],
                        start=(ci == 0),
                        stop=(ci == CK - 1),
                    )
            for fi in range(3):
                nc.vector.tensor_copy(
                    out=dest[fi][b][:, tcol * TB : (tcol + 1) * TB], in_=pss[fi][:]
                )

            # As soon as a batch's vT is complete, build its V-natural tiles
            # (PE transpose of 64-row slices through the identity).
            if tcol == NTB // B - 1:
                for h in range(HPC):
                    hp = slice(h * 64, (h + 1) * 64)
                    for kti in range(NKT):
                        tr = psA.tile([128, 64], F16, name="vtr", tag="psA")
                        nc.tensor.transpose(
                            tr[:], vT[b][hp, kti * 128 : (kti + 1) * 128], ident[hp, hp]
                        )
                        nc.vector.tensor_copy(out=V1[:, b, h, kti, 0:64], in_=tr[:])

        for tb in range(NTB // B):
            qkv_block(tb)

        # ---------- phase 2: causal attention ----------
        # Both heads interleaved per (b, qb) and PV software-pipelined one kt
        # tile behind the scores so the PE never stalls on the ACT exp.
        # Unnormalized [PV | sums] results are copied to SBUF (freeing PSUM)
        # and all 16 sum-rows are collected so one batched reciprocal covers
        # the whole kernel (a [1, N] DVE reciprocal is ~3.4 us — single lane).
        pending = []
        for tb in range(NTB // B, NTB):
            pending.append(lambda tb=tb: qkv_block(tb))

        def emit_pending(n=None):
            cnt = len(pending) if n is None else min(n, len(pending))
            for _ in range(cnt):
                pending.pop(0)()

        for b in range(B):
            # batch 1's attention consumes batch-1 QKV/V1: those must be
            # emitted (Tile dep-tracking follows emission order) before it.
            if b == 1:
                emit_pending()
            for qb in range(NQB):
                nkt = 4 * qb + 4
                pv = [
                    psPV.tile([65, TB], F32, name=f"pv_ps{h}", tag=f"psPV{h}")
                    for h in range(HPC)
                ]
                stages = []  # deferred PV matmuls, one kti behind the scores

                def flush(n=None):
                    while stages and (n is None or len(stages) > n):
                        stages.pop(0)()

                for kti in range(nkt):
                    qs = max(0, kti * 128 - qb * TB)  # local col start
                    N = TB - qs
                    # both heads' scores in one 2-bank PSUM tile -> one exp
                    sps = psA.tile([128, HPC, TB], F32, name="s_ps", tag="psA")
                    for h in range(HPC):
                        hp = slice(h * 64, (h + 1) * 64)
                        nc.tensor.matmul(
                            sps[:, h, 0:N],
                            kT[b][hp, kti * 128 : (kti + 1) * 128],
                            qT[b][hp, qb * TB + qs : (qb + 1) * TB],
                            start=True,
                            stop=True,
                        )
                    P = ppool.tile([128, HPC, TB], F16, name="Pt", tag="P")
                    nc.scalar.activation(
                        P[:, :, 0:N],
                        sps[:, :, 0:N],
                        mybir.ActivationFunctionType.Exp,
                        scale=SCALE,
                    )
                    if kti * 128 >= qb * TB:
                        # diagonal tile: first 128 cols of each head hold the
                        # triangle; one DVE mult covers both heads
                        nc.vector.tensor_mul(
                            P[:, :, 0:128], P[:, :, 0:128], trimask2[:]
                        )

                    def pv_step(kti=kti, qs=qs, N=N, P=P):
                        for h in range(HPC):
                            nc.tensor.matmul(
                                pv[h][:, qs:TB],
                                V1[:, b, h, kti, :],
                                P[:, h, 0:N],
                                start=(kti == 0),
                                stop=(kti == nkt - 1),
                            )

                    stages.append(pv_step)
                    flush(1)
                    if kti in (3, 6):
                        emit_pending(1)
                flush()

                # normalize this q-block inline (reciprocal_approx_fast is
                # ~18-bit accurate, plenty above the fp32r noise floor), then
                # emit its projection: the proj matmuls are exp-independent
                # PE work that fills the next q-block's ACT stalls.
                for h in range(HPC):
                    hp = slice(h * 64, (h + 1) * 64)
                    pvt = pvpool.tile([65, TB], F32, name="pvt", tag="pvt")
                    nc.vector.tensor_copy(out=pvt[:], in_=pv[h][:])
                    # custom-DVE ops require partition-0 sources on HW; plain
                    # copies handle the 64->0 partition shift fine.
                    s0 = npool.tile([1, TB], F32, name="s0", tag="s0")
                    nc.vector.tensor_copy(out=s0[:], in_=pvt[64:65, :])
                    rt = npool.tile([1, TB], F32, name="rt", tag="rt")
                    nc.vector.reciprocal_approx_fast(rt[:], s0[:])
                    bc = npool.tile([64, TB], F32, name="bc", tag="bc")
                    nc.gpsimd.partition_broadcast(bc[:], rt[:])
                    nc.vector.tensor_mul(
                        attnT[b][hp, qb * TB : (qb + 1) * TB], pvt[0:64, :], bc[:]
                    )
                def proj_step(b=b, qb=qb):
                    for ti in range(4 * qb, 4 * qb + 4):
                        for fb in range(C // TB):
                            ps = psA.tile([128, TB], F32, name="y_ps", tag="psA")
                            nc.tensor.matmul(
                                ps[:],
                                attnT[b][:, ti * 128 : (ti + 1) * 128],
                                wp_sb[:, fb * TB : (fb + 1) * TB],
                                start=True,
                                stop=True,
                            )
                            ysb = ypool.tile([128, TB], F32, name="ysb", tag="ysb")
                            nc.vector.tensor_copy(out=ysb[:], in_=ps[:])
                            nc.sync.dma_start(
                                y[b * T + ti * 128 : b * T + (ti + 1) * 128,
                                  fb * TB : (fb + 1) * TB],
                                ysb[:],
                            )

                pending.append(proj_step)
        emit_pending()
    nc.compile()
    return nc


def make_in_maps(x, w_attn, w_proj):
    """Host-side sharding into the per-core layouts."""
    x = np.asarray(x, dtype=np.float32)
    w_attn = np.asarray(w_attn, dtype=np.float32)
    w_proj = np.asarray(w_proj, dtype=np.float32)

    xT = np.ascontiguousarray(x.reshape(BT, C).T.astype(np.float16))
    wpT_full = np.ascontiguousarray(w_proj.T.astype(np.float16))

    in_maps = []
    for c in range(NCORES):
        rows = []
        for sec in range(3):                                # q, k, v
            for h in (HPC * c, HPC * c + 1):
                rows.extend(range(sec * C + h * D, sec * C + (h + 1) * D))
        wqkvT = np.ascontiguousarray(w_attn[rows, :].T.astype(np.float16))
        wpT = np.ascontiguousarray(
            wpT_full[c * HPC * D : (c + 1) * HPC * D, :]    # [128, 1024]
        )
        consts = np.stack(
            [
                np.eye(128, dtype=np.float16),
                np.tril(np.ones((128, 128), np.float16)).T,  # keep kt <= qt
            ]
        )
        in_maps.append({"xT": xT, "wqkvT": wqkvT, "wpT": wpT, "consts": consts})
    return in_maps


_PROGRAM = None


def _program():
    global _PROGRAM
    if _PROGRAM is None:
        _PROGRAM = build_program()
    return _PROGRAM


def kernel(x, w_attn, w_proj):
    from concourse.bass_utils import run_bass_kernel_spmd

    res = run_bass_kernel_spmd(
        _program(), make_in_maps(x, w_attn, w_proj), list(range(NCORES))
    )
    out = res.results[0]["y"].astype(np.float32, copy=True)
    for i in range(1, NCORES):
        out += res.results[i]["y"]
    return out.reshape(B, T, C)


# revision 31
# speedup vs baseline: 1.3407x; 1.0030x over previous
"""Causal self-attention on 8 Trainium2 NeuronCores.

Sharding: 2 heads per core (tensor parallel).  The host pre-transposes the
activations/weights into the layouts the PE array wants, each core computes
QKV -> causal attention -> its partial of the output projection for its two
heads, and the host sums the 8 partial projections (row-parallel linear).

Per-core device program (SPMD, different data per core):
  xT    [1024, 4096]  x transposed, rows=embed c, cols=token t (t = b*2048+tt)
  wqkvT [1024, 384]   w_attn rows for this core's heads, transposed.
                      f = [q_h0 d0..63 | q_h1 | k_h0 | k_h1 | v_h0 | v_h1]
  wpT   [128, 1024]   w_proj columns for this core's channels, transposed
  y     [4096, 1024]  partial output (sum over cores = final)

Dataflow (everything "transposed" so the PE contraction dim is the partition
dim with no on-device transposes of activations):
  qkvT[f, t]   = wqkvT_tile.T @ xT_tile            (accumulate over 8 c-tiles)
  S^T[kt, qt]  = kT_tile.T @ qT_block              (K = head dim 64)
  P^T          = exp(S^T / 32)                     (ACT; no max subtraction --
                                                    scores are O(1), exp safe)
  causal mask  = multiply diagonal 128x128 block by 0/1 lower-tri tile
  outT[65,qt] += [V | ones].T @ P^T                (row 64 = softmax sums)
  attnT        = outT[0:64] * (1 / outT[64])       (broadcast along partitions)
  y[t, f]      = attnT_tile.T @ wpT                (partial; host sums cores)

All matmul operands are float16 (host-converted): 2-byte operands stream at
1 PE cycle/row (fp32/fp32r is SBUF-read-bandwidth-bound at ~2 cycles/row) and
halve the HBM traffic; accumulation stays fp32 in PSUM, the final projection
output and the host-side cross-core sum are fp32.  Measured end-to-end
relative error vs the fp32 reference: ~5e-4.

Measured on 8 axon trn2 cores: ~272 us HW exec (from a 395 us fp32r
baseline).  Key scheduling facts baked in: Tile's PE order is static, so the
projection of q-block i is *emitted* inside q-block i+1's score loop to fill
the PE stalls left by the ACT exp; PV matmuls trail scores by one kt tile;
softmax uses no max-subtraction (scores are O(1)); normalization is
reciprocal_approx_fast + gpsimd partition_broadcast (source must sit at
partition 0 on HW).
"""

import numpy as np

B, T, C = 2, 2048, 1024
H, D = 16, 64
NCORES = 8
HPC = H // NCORES          # heads per core = 2
BT = B * T                 # 4096 tokens total
TB = 512                   # token block (matmul moving free dim)
CK = C // 128              # 8 contraction tiles for the projections
NTB = BT // TB             # 8 token blocks
NQB = T // TB              # 4 q blocks per batch
NKT = T // 128             # 16 kt tiles per batch
SCALE = 1.0 / 32.0         # 1 / sqrt(C)


def build_program():
    """Build the single-core Bass program (same program runs on all 8 cores)."""
    from contextlib import ExitStack

    import concourse.mybir as mybir
    import concourse.tile as tile
    from concourse import bacc, library_config

    dt = mybir.dt
    F32 = dt.float32
    F32R = dt.float32r
    F16 = dt.float16

    nc = bacc.Bacc("TRN2")
    xT = nc.dram_tensor("xT", [C, BT], F16, kind="ExternalInput").ap()
    wqkvT = nc.dram_tensor("wqkvT", [C, 3 * HPC * D], F16, kind="ExternalInput").ap()
    wpT = nc.dram_tensor("wpT", [HPC * D, C], F16, kind="ExternalInput").ap()
    # consts[0] = 128x128 identity, consts[1] = causal keep-mask
    # (mask[kt, qt] = 1.0 where kt <= qt)
    consts = nc.dram_tensor("consts", [2, 128, 128], F16, kind="ExternalInput").ap()
    y = nc.dram_tensor("y", [BT, C], F32, kind="ExternalOutput").ap()

    with ExitStack() as ctx:
        tc = ctx.enter_context(tile.TileContext(nc))
        const = ctx.enter_context(tc.tile_pool(name="const", bufs=1))
        xpool = ctx.enter_context(tc.tile_pool(name="xload", bufs=16))
        ppool = ctx.enter_context(tc.tile_pool(name="pexp", bufs=6))
        npool = ctx.enter_context(tc.tile_pool(name="norm", bufs=6))
        pvpool = ctx.enter_context(tc.tile_pool(name="pvs", bufs=4))
        ypool = ctx.enter_context(tc.tile_pool(name="yout", bufs=4))
        psA = ctx.enter_context(tc.tile_pool(name="psA", bufs=2, space="PSUM"))
        psPV = ctx.enter_context(tc.tile_pool(name="psPV", bufs=2, space="PSUM"))

        # ---------- constants / persistent SBUF ----------
        w_sb = const.tile([128, CK, 3 * HPC * D], F16, name="w_sb")
        wqkvT_t = wqkvT.rearrange("(a p) f -> p a f", p=128)
        for ci in range(CK):
            nc.sync.dma_start(w_sb[:, ci, :], wqkvT_t[:, ci, :])
        wp_sb = const.tile([128, C], F16, name="wp_sb")
        ident = const.tile([128, 128], F16, name="ident")
        trimask2 = const.tile([128, HPC, 128], F16, name="trimask2")

        def load_consts():
            nc.sync.dma_start(wp_sb[:], wpT)
            nc.sync.dma_start(ident[:], consts[0])
            for _h in range(HPC):
                nc.sync.dma_start(trimask2[:, _h, :], consts[1])

        # partition_broadcast lives in the "attn" GPSIMD library; same-engine
        # FIFO order guarantees this lands before the broadcasts.
        nc.gpsimd.load_library(library_config.attn)

        # Per-batch transposed activations, heads packed on partitions
        # (h0 -> partitions 0:64, h1 -> 64:128).
        qT = [const.tile([128, T], F16, name=f"qT{b}") for b in range(B)]
        kT = [const.tile([128, T], F16, name=f"kT{b}") for b in range(B)]
        vT = [const.tile([128, T], F16, name=f"vT{b}") for b in range(B)]
        attnT = [const.tile([128, T], F16, name=f"attnT{b}") for b in range(B)]

        # [V | ones] stationary tiles for PV: V1[:, b, h, kti, 0:64] = V natural
        # [kt, d]; column 64 = 1.0 so PV row 64 accumulates the softmax sums.
        V1 = const.tile([128, B, HPC, NKT, 65], F16, name="V1")
        nc.vector.memset(V1[:, :, :, :, 64:65], 1.0)

        # ---------- phase 1: QKV projection ----------
        dest = {0: qT, 1: kT, 2: vT}

        def qkv_block(tb):
            b, tcol = divmod(tb, NTB // B)
            xts = []
            for ci in range(CK):
                xt = xpool.tile([128, TB], F16, name="xt", tag="xt")
                nc.sync.dma_start(
                    xt[:],
                    xT[ci * 128 : (ci + 1) * 128, tb * TB : (tb + 1) * TB],
                )
                xts.append(xt)
            qtags = ["psA", "psPV0", "psPV1"]
            pss = [
                psA.tile([128, TB], F32, name="qkv_ps", tag="psA")
                if fi == 0
                else psPV.tile([128, TB], F32, name=f"qkv_ps{fi}", tag=qtags[fi])
                for fi in range(3)
            ]
            for ci in range(CK):
                for fi in range(3):
                    nc.tensor.matmul(
                        pss[fi][:],
                        w_sb[:, ci, fi * 128 : (fi + 1) * 128],
                        xts[ci][:],
                        start=(ci == 0),
                        stop=(ci == CK - 1),
                    )
            for fi in range(3):
                nc.vector.tensor_copy(
                    out=dest[fi][b][:, tcol * TB : (tcol + 1) * TB], in_=pss[fi][:]
                )

            # As soon as a batch's vT is complete, build its V-natural tiles
            # (PE transpose of 64-row slices through the identity).
            if tcol == NTB // B - 1:
                for h in range(HPC):
                    hp = slice(h * 64, (h + 1) * 64)
                    for kti in range(NKT):
                        tr = psA.tile([128, 64], F16, name="vtr", tag="psA")
                        nc.tensor.transpose(
                            tr[:], vT[b][hp, kti * 128 : (kti + 1) * 128], ident[hp, hp]
                        )
                        nc.vector.tensor_copy(out=V1[:, b, h, kti, 0:64], in_=tr[:])

        qkv_block(0)
        load_consts()  # behind tb0's x tiles on the DMA FIFO, ahead of their use
        for tb in range(1, NTB // B):
            qkv_block(tb)

        # ---------- phase 2: causal attention ----------
        # Both heads interleaved per (b, qb) and PV software-pipelined one kt
        # tile behind the scores so the PE never stalls on the ACT exp.
        # Unnormalized [PV | sums] results are copied to SBUF (freeing PSUM)
        # and all 16 sum-rows are collected so one batched reciprocal covers
        # the whole kernel (a [1, N] DVE reciprocal is ~3.4 us — single lane).
        pending = []
        for tb in range(NTB // B, NTB):
            pending.append(lambda tb=tb: qkv_block(tb))

        def emit_pending(n=None):
            cnt = len(pending) if n is None else min(n, len(pending))
            for _ in range(cnt):
                pending.pop(0)()

        for b in range(B):
            # batch 1's attention consumes batch-1 QKV/V1: those must be
            # emitted (Tile dep-tracking follows emission order) before it.
            if b == 1:
                emit_pending()
            for qb in range(NQB):
                nkt = 4 * qb + 4
                pv = [
                    psPV.tile([65, TB], F32, name=f"pv_ps{h}", tag=f"psPV{h}")
                    for h in range(HPC)
                ]
                stages = []  # deferred PV matmuls, one kti behind the scores

                def flush(n=None):
                    while stages and (n is None or len(stages) > n):
                        stages.pop(0)()

                for kti in range(nkt):
                    qs = max(0, kti * 128 - qb * TB)  # local col start
                    N = TB - qs
                    # both heads' scores in one 2-bank PSUM tile -> one exp
                    sps = psA.tile([128, HPC, TB], F32, name="s_ps", tag="psA")
                    for h in range(HPC):
                        hp = slice(h * 64, (h + 1) * 64)
                        nc.tensor.matmul(
                            sps[:, h, 0:N],
                            kT[b][hp, kti * 128 : (kti + 1) * 128],
                            qT[b][hp, qb * TB + qs : (qb + 1) * TB],
                            start=True,
                            stop=True,
                        )
                    P = ppool.tile([128, HPC, TB], F16, name="Pt", tag="P")
                    nc.scalar.activation(
                        P[:, :, 0:N],
                        sps[:, :, 0:N],
                        mybir.ActivationFunctionType.Exp,
                        scale=SCALE,
                    )
                    if kti * 128 >= qb * TB:
                        # diagonal tile: first 128 cols of each head hold the
                        # triangle; one DVE mult covers both heads
                        nc.vector.tensor_mul(
                            P[:, :, 0:128], P[:, :, 0:128], trimask2[:]
                        )

                    def pv_step(kti=kti, qs=qs, N=N, P=P):
                        for h in range(HPC):
                            nc.tensor.matmul(
                                pv[h][:, qs:TB],
                                V1[:, b, h, kti, :],
                                P[:, h, 0:N],
                                start=(kti == 0),
                                stop=(kti == nkt - 1),
                            )

                    stages.append(pv_step)
                    flush(1)
                    if kti in (3, 6):
                        emit_pending(1)
                flush()

                # normalize this q-block inline (reciprocal_approx_fast is
                # ~18-bit accurate, plenty above the fp32r noise floor), then
                # emit its projection: the proj matmuls are exp-independent
                # PE work that fills the next q-block's ACT stalls.
                for h in range(HPC):
                    hp = slice(h * 64, (h + 1) * 64)
                    pvt = pvpool.tile([65, TB], F32, name="pvt", tag="pvt")
                    nc.vector.tensor_copy(out=pvt[:], in_=pv[h][:])
                    # custom-DVE ops require partition-0 sources on HW; plain
                    # copies handle the 64->0 partition shift fine.
                    s0 = npool.tile([1, TB], F32, name="s0", tag="s0")
                    nc.vector.tensor_copy(out=s0[:], in_=pvt[64:65, :])
                    rt = npool.tile([1, TB], F32, name="rt", tag="rt")
                    nc.vector.reciprocal_approx_fast(rt[:], s0[:])
                    bc = npool.tile([64, TB], F32, name="bc", tag="bc")
                    nc.gpsimd.partition_broadcast(bc[:], rt[:])
                    nc.vector.tensor_mul(
                        attnT[b][hp, qb * TB : (qb + 1) * TB], pvt[0:64, :], bc[:]
                    )
                def proj_step(b=b, qb=qb):
                    for ti in range(4 * qb, 4 * qb + 4):
                        for fb in range(C // TB):
                            ps = psA.tile([128, TB], F32, name="y_ps", tag="psA")
                            nc.tensor.matmul(
                                ps[:],
                                attnT[b][:, ti * 128 : (ti + 1) * 128],
                                wp_sb[:, fb * TB : (fb + 1) * TB],
                                start=True,
                                stop=True,
                            )
                            ysb = ypool.tile([128, TB], F32, name="ysb", tag="ysb")
                            nc.vector.tensor_copy(out=ysb[:], in_=ps[:])
                            nc.sync.dma_start(
                                y[b * T + ti * 128 : b * T + (ti + 1) * 128,
                                  fb * TB : (fb + 1) * TB],
                                ysb[:],
                            )

                pending.append(proj_step)
        emit_pending()
    nc.compile()
    return nc


def make_in_maps(x, w_attn, w_proj):
    """Host-side sharding into the per-core layouts."""
    x = np.asarray(x, dtype=np.float32)
    w_attn = np.asarray(w_attn, dtype=np.float32)
    w_proj = np.asarray(w_proj, dtype=np.float32)

    xT = np.ascontiguousarray(x.reshape(BT, C).T.astype(np.float16))
    wpT_full = np.ascontiguousarray(w_proj.T.astype(np.float16))

    in_maps = []
    for c in range(NCORES):
        rows = []
        for sec in range(3):                                # q, k, v
            for h in (HPC * c, HPC * c + 1):
                rows.extend(range(sec * C + h * D, sec * C + (h + 1) * D))
        wqkvT = np.ascontiguousarray(w_attn[rows, :].T.astype(np.float16))
        wpT = np.ascontiguousarray(
            wpT_full[c * HPC * D : (c + 1) * HPC * D, :]    # [128, 1024]
        )
        consts = np.stack(
            [
                np.eye(128, dtype=np.float16),
                np.tril(np.ones((128, 128), np.float16)).T,  # keep kt <= qt
            ]
        )
        in_maps.append({"xT": xT, "wqkvT": wqkvT, "wpT": wpT, "consts": consts})
    return in_maps


_PROGRAM = None


def _program():
    global _PROGRAM
    if _PROGRAM is None:
        _PROGRAM = build_program()
    return _PROGRAM


def kernel(x, w_attn, w_proj):
    from concourse.bass_utils import run_bass_kernel_spmd

    res = run_bass_kernel_spmd(
        _program(), make_in_maps(x, w_attn, w_proj), list(range(NCORES))
    )
    out = res.results[0]["y"].astype(np.float32, copy=True)
    for i in range(1, NCORES):
        out += res.results[i]["y"]
    return out.reshape(B, T, C)


# revision 32
# speedup vs baseline: 1.3540x; 1.0100x over previous
"""Causal self-attention on 8 Trainium2 NeuronCores.

Sharding: 2 heads per core (tensor parallel).  The host pre-transposes the
activations/weights into the layouts the PE array wants, each core computes
QKV -> causal attention -> its partial of the output projection for its two
heads, and the host sums the 8 partial projections (row-parallel linear).

Per-core device program (SPMD, different data per core):
  xT    [1024, 4096]  x transposed, rows=embed c, cols=token t (t = b*2048+tt)
  wqkvT [1024, 384]   w_attn rows for this core's heads, transposed.
                      f = [q_h0 d0..63 | q_h1 | k_h0 | k_h1 | v_h0 | v_h1]
  wpT   [128, 1024]   w_proj columns for this core's channels, transposed
  y     [4096, 1024]  partial output (sum over cores = final)

Dataflow (everything "transposed" so the PE contraction dim is the partition
dim with no on-device transposes of activations):
  qkvT[f, t]   = wqkvT_tile.T @ xT_tile            (accumulate over 8 c-tiles)
  S^T[kt, qt]  = kT_tile.T @ qT_block              (K = head dim 64)
  P^T          = exp(S^T / 32)                     (ACT; no max subtraction --
                                                    scores are O(1), exp safe)
  causal mask  = multiply diagonal 128x128 block by 0/1 lower-tri tile
  outT[65,qt] += [V | ones].T @ P^T                (row 64 = softmax sums)
  attnT        = outT[0:64] * (1 / outT[64])       (broadcast along partitions)
  y[t, f]      = attnT_tile.T @ wpT                (partial; host sums cores)

All matmul operands are float16 (host-converted): 2-byte operands stream at
1 PE cycle/row (fp32/fp32r is SBUF-read-bandwidth-bound at ~2 cycles/row) and
halve the HBM traffic; accumulation stays fp32 in PSUM, the final projection
output and the host-side cross-core sum are fp32.  Measured end-to-end
relative error vs the fp32 reference: ~5e-4.

Measured on 8 axon trn2 cores: ~272 us HW exec (from a 395 us fp32r
baseline).  Key scheduling facts baked in: Tile's PE order is static, so the
projection of q-block i is *emitted* inside q-block i+1's score loop to fill
the PE stalls left by the ACT exp; PV matmuls trail scores by one kt tile;
softmax uses no max-subtraction (scores are O(1)); normalization is
reciprocal_approx_fast + gpsimd partition_broadcast (source must sit at
partition 0 on HW).
"""

import numpy as np

B, T, C = 2, 2048, 1024
H, D = 16, 64
NCORES = 8
HPC = H // NCORES          # heads per core = 2
BT = B * T                 # 4096 tokens total
TB = 512                   # token block (matmul moving free dim)
CK = C // 128              # 8 contraction tiles for the projections
NTB = BT // TB             # 8 token blocks
NQB = T // TB              # 4 q blocks per batch
NKT = T // 128             # 16 kt tiles per batch
SCALE = 1.0 / 32.0         # 1 / sqrt(C)


def build_program():
    """Build the single-core Bass program (same program runs on all 8 cores)."""
    from contextlib import ExitStack

    import concourse.mybir as mybir
    import concourse.tile as tile
    from concourse import bacc, library_config

    dt = mybir.dt
    F32 = dt.float32
    F32R = dt.float32r
    F16 = dt.float16

    nc = bacc.Bacc("TRN2")
    xT = nc.dram_tensor("xT", [C, BT], F16, kind="ExternalInput").ap()
    wqkvT = nc.dram_tensor("wqkvT", [C, 3 * HPC * D], F16, kind="ExternalInput").ap()
    wpT = nc.dram_tensor("wpT", [HPC * D, C], F16, kind="ExternalInput").ap()
    # consts[0] = 128x128 identity, consts[1] = causal keep-mask
    # (mask[kt, qt] = 1.0 where kt <= qt)
    consts = nc.dram_tensor("consts", [2, 128, 128], F16, kind="ExternalInput").ap()
    y = nc.dram_tensor("y", [BT, C], F32, kind="ExternalOutput").ap()

    with ExitStack() as ctx:
        tc = ctx.enter_context(tile.TileContext(nc))
        const = ctx.enter_context(tc.tile_pool(name="const", bufs=1))
        xpool = ctx.enter_context(tc.tile_pool(name="xload", bufs=16))
        ppool = ctx.enter_context(tc.tile_pool(name="pexp", bufs=6))
        npool = ctx.enter_context(tc.tile_pool(name="norm", bufs=6))
        pvpool = ctx.enter_context(tc.tile_pool(name="pvs", bufs=4))
        ypool = ctx.enter_context(tc.tile_pool(name="yout", bufs=4))
        psA = ctx.enter_context(tc.tile_pool(name="psA", bufs=2, space="PSUM"))
        psPV = ctx.enter_context(tc.tile_pool(name="psPV", bufs=2, space="PSUM"))

        # ---------- constants / persistent SBUF ----------
        w_sb = const.tile([128, CK, 3 * HPC * D], F16, name="w_sb")
        wqkvT_t = wqkvT.rearrange("(a p) f -> p a f", p=128)
        for ci in range(CK):
            nc.sync.dma_start(w_sb[:, ci, :], wqkvT_t[:, ci, :])
        wp_sb = const.tile([128, C], F16, name="wp_sb")
        ident = const.tile([128, 128], F16, name="ident")
        trimask2 = const.tile([128, HPC, 128], F16, name="trimask2")

        def load_consts():
            nc.sync.dma_start(wp_sb[:], wpT)
            nc.sync.dma_start(ident[:], consts[0])
            for _h in range(HPC):
                nc.sync.dma_start(trimask2[:, _h, :], consts[1])

        # partition_broadcast lives in the "attn" GPSIMD library; same-engine
        # FIFO order guarantees this lands before the broadcasts.
        nc.gpsimd.load_library(library_config.attn)

        # Per-batch transposed activations, heads packed on partitions
        # (h0 -> partitions 0:64, h1 -> 64:128).
        qT = [const.tile([128, T], F16, name=f"qT{b}") for b in range(B)]
        kT = [const.tile([128, T], F16, name=f"kT{b}") for b in range(B)]
        vT = [const.tile([128, T], F16, name=f"vT{b}") for b in range(B)]
        attnT = [const.tile([128, T], F16, name=f"attnT{b}") for b in range(B)]

        # [V | ones] stationary tiles for PV: V1[:, b, h, kti, 0:64] = V natural
        # [kt, d]; column 64 = 1.0 so PV row 64 accumulates the softmax sums.
        V1 = const.tile([128, B, HPC, NKT, 65], F16, name="V1")
        nc.vector.memset(V1[:, :, :, :, 64:65], 1.0)

        # ---------- phase 1: QKV projection ----------
        dest = {0: qT, 1: kT, 2: vT}

        def qkv_block(tb):
            b, tcol = divmod(tb, NTB // B)
            xts = []
            for ci in range(CK):
                xt = xpool.tile([128, TB], F16, name="xt", tag="xt")
                nc.sync.dma_start(
                    xt[:],
                    xT[ci * 128 : (ci + 1) * 128, tb * TB : (tb + 1) * TB],
                )
                xts.append(xt)
            qtags = ["psA", "psPV0", "psPV1"]
            pss = [
                psA.tile([128, TB], F32, name="qkv_ps", tag="psA")
                if fi == 0
                else psPV.tile([128, TB], F32, name=f"qkv_ps{fi}", tag=qtags[fi])
                for fi in range(3)
            ]
            for ci in range(CK):
                for fi in range(3):
                    nc.tensor.matmul(
                        pss[fi][:],
                        w_sb[:, ci, fi * 128 : (fi + 1) * 128],
                        xts[ci][:],
                        start=(ci == 0),
                        stop=(ci == CK - 1),
                    )
            for fi in range(3):
                nc.vector.tensor_copy(
                    out=dest[fi][b][:, tcol * TB : (tcol + 1) * TB], in_=pss[fi][:]
                )

            # As soon as a batch's vT is complete, build its V-natural tiles
            # (PE transpose of 64-row slices through the identity).
            if tcol == NTB // B - 1:
                for h in range(HPC):
                    hp = slice(h * 64, (h + 1) * 64)
                    for kti in range(NKT):
                        tr = psA.tile([128, 64], F16, name="vtr", tag="psA")
                        nc.tensor.transpose(
                            tr[:], vT[b][hp, kti * 128 : (kti + 1) * 128], ident[hp, hp]
                        )
                        nc.vector.tensor_copy(out=V1[:, b, h, kti, 0:64], in_=tr[:])

        qkv_block(0)
        load_consts()  # behind tb0's x tiles on the DMA FIFO, ahead of their use
        for tb in range(1, NTB // B):
            qkv_block(tb)

        # ---------- phase 2: causal attention ----------
        # Both heads interleaved per (b, qb) and PV software-pipelined one kt
        # tile behind the scores so the PE never stalls on the ACT exp.
        # Unnormalized [PV | sums] results are copied to SBUF (freeing PSUM)
        # and all 16 sum-rows are collected so one batched reciprocal covers
        # the whole kernel (a [1, N] DVE reciprocal is ~3.4 us — single lane).
        pending = []
        for tb in range(NTB // B, NTB):
            pending.append(("qkv", lambda tb=tb: qkv_block(tb)))

        def emit_pending(n=None):
            cnt = len(pending) if n is None else min(n, len(pending))
            for _ in range(cnt):
                pending.pop(0)[1]()

        def emit_pending_qkv():
            # only QKV closures must precede batch 1's attention emission;
            # a deferred proj flushed here would stall on its fresh norm chain
            while any(k == "qkv" for k, _ in pending):
                pending.pop(0)[1]()

        for b in range(B):
            # batch 1's attention consumes batch-1 QKV/V1: those must be
            # emitted (Tile dep-tracking follows emission order) before it.
            if b == 1:
                emit_pending_qkv()
            for qb in range(NQB):
                nkt = 4 * qb + 4
                pv = [
                    psPV.tile([65, TB], F32, name=f"pv_ps{h}", tag=f"psPV{h}")
                    for h in range(HPC)
                ]
                stages = []  # deferred PV matmuls, one kti behind the scores

                def flush(n=None):
                    while stages and (n is None or len(stages) > n):
                        stages.pop(0)()

                for kti in range(nkt):
                    qs = max(0, kti * 128 - qb * TB)  # local col start
                    N = TB - qs
                    # both heads' scores in one 2-bank PSUM tile -> one exp
                    sps = psA.tile([128, HPC, TB], F32, name="s_ps", tag="psA")
                    for h in range(HPC):
                        hp = slice(h * 64, (h + 1) * 64)
                        nc.tensor.matmul(
                            sps[:, h, 0:N],
                            kT[b][hp, kti * 128 : (kti + 1) * 128],
                            qT[b][hp, qb * TB + qs : (qb + 1) * TB],
                            start=True,
                            stop=True,
                        )
                    P = ppool.tile([128, HPC, TB], F16, name="Pt", tag="P")
                    nc.scalar.activation(
                        P[:, :, 0:N],
                        sps[:, :, 0:N],
                        mybir.ActivationFunctionType.Exp,
                        scale=SCALE,
                    )
                    if kti * 128 >= qb * TB:
                        # diagonal tile: first 128 cols of each head hold the
                        # triangle; one DVE mult covers both heads
                        nc.vector.tensor_mul(
                            P[:, :, 0:128], P[:, :, 0:128], trimask2[:]
                        )

                    def pv_step(kti=kti, qs=qs, N=N, P=P):
                        for h in range(HPC):
                            nc.tensor.matmul(
                                pv[h][:, qs:TB],
                                V1[:, b, h, kti, :],
                                P[:, h, 0:N],
                                start=(kti == 0),
                                stop=(kti == nkt - 1),
                            )

                    stages.append(pv_step)
                    flush(1)
                    if kti in (3, 6):
                        emit_pending(1)
                flush()

                # normalize this q-block inline (reciprocal_approx_fast is
                # ~18-bit accurate, plenty above the fp32r noise floor), then
                # emit its projection: the proj matmuls are exp-independent
                # PE work that fills the next q-block's ACT stalls.
                for h in range(HPC):
                    hp = slice(h * 64, (h + 1) * 64)
                    pvt = pvpool.tile([65, TB], F32, name="pvt", tag="pvt")
                    nc.vector.tensor_copy(out=pvt[:], in_=pv[h][:])
                    # custom-DVE ops require partition-0 sources on HW; plain
                    # copies handle the 64->0 partition shift fine.
                    s0 = npool.tile([1, TB], F32, name="s0", tag="s0")
                    nc.vector.tensor_copy(out=s0[:], in_=pvt[64:65, :])
                    rt = npool.tile([1, TB], F32, name="rt", tag="rt")
                    nc.vector.reciprocal_approx_fast(rt[:], s0[:])
                    bc = npool.tile([64, TB], F32, name="bc", tag="bc")
                    nc.gpsimd.partition_broadcast(bc[:], rt[:])
                    nc.vector.tensor_mul(
                        attnT[b][hp, qb * TB : (qb + 1) * TB], pvt[0:64, :], bc[:]
                    )
                def proj_step(b=b, qb=qb):
                    for ti in range(4 * qb, 4 * qb + 4):
                        for fb in range(C // TB):
                            ps = psA.tile([128, TB], F32, name="y_ps", tag="psA")
                            nc.tensor.matmul(
                                ps[:],
                                attnT[b][:, ti * 128 : (ti + 1) * 128],
                                wp_sb[:, fb * TB : (fb + 1) * TB],
                                start=True,
                                stop=True,
                            )
                            ysb = ypool.tile([128, TB], F32, name="ysb", tag="ysb")
                            nc.vector.tensor_copy(out=ysb[:], in_=ps[:])
                            nc.sync.dma_start(
                                y[b * T + ti * 128 : b * T + (ti + 1) * 128,
                                  fb * TB : (fb + 1) * TB],
                                ysb[:],
                            )

                pending.append(("proj", proj_step))
        emit_pending()
    nc.compile()
    return nc


def make_in_maps(x, w_attn, w_proj):
    """Host-side sharding into the per-core layouts."""
    x = np.asarray(x, dtype=np.float32)
    w_attn = np.asarray(w_attn, dtype=np.float32)
    w_proj = np.asarray(w_proj, dtype=np.float32)

    xT = np.ascontiguousarray(x.reshape(BT, C).T.astype(np.float16))
    wpT_full = np.ascontiguousarray(w_proj.T.astype(np.float16))

    in_maps = []
    for c in range(NCORES):
        rows = []
        for sec in range(3):                                # q, k, v
            for h in (HPC * c, HPC * c + 1):
                rows.extend(range(sec * C + h * D, sec * C + (h + 1) * D))
        wqkvT = np.ascontiguousarray(w_attn[rows, :].T.astype(np.float16))
        wpT = np.ascontiguousarray(
            wpT_full[c * HPC * D : (c + 1) * HPC * D, :]    # [128, 1024]
        )
        consts = np.stack(
            [
                np.eye(128, dtype=np.float16),
                np.tril(np.ones((128, 128), np.float16)).T,  # keep kt <= qt
            ]
        )
        in_maps.append({"xT": xT, "wqkvT": wqkvT, "wpT": wpT, "consts": consts})
    return in_maps


_PROGRAM = None


def _program():
    global _PROGRAM
    if _PROGRAM is None:
        _PROGRAM = build_program()
    return _PROGRAM


def kernel(x, w_attn, w_proj):
    from concourse.bass_utils import run_bass_kernel_spmd

    res = run_bass_kernel_spmd(
        _program(), make_in_maps(x, w_attn, w_proj), list(range(NCORES))
    )
    out = res.results[0]["y"].astype(np.float32, copy=True)
    for i in range(1, NCORES):
        out += res.results[i]["y"]
    return out.reshape(B, T, C)
